# revision 10
# baseline (speedup 1.0000x reference)
"""Block-sparse attention (SageAttention-style mean-similarity top-k) on 8 TRN2 NeuronCores.

Sharding: 16 heads tensor-parallel across 8 cores (2 heads/core).
  - qkv weight column-sharded per core (its 2 heads' q/k/v rows, pre-transposed on host)
  - attention (block selection + block-sparse softmax-V) fully local per head
  - attention outputs AllGathered (token-major bf16), proj weight row-sharded:
    each core computes a 128-column slice of the output; host concatenates.

Per-core device pipeline (bf16 compute, f32 selection):
  x^T f32 -> block sums (DVE) -> qm/km/sim (f32 PE) -> top-16 via max8/max_index
  qkv matmuls (bf16 PE), k kept d-major, v token-major with a ones column
  per query block: ap_gather (GPSIMD ucode, SBUF->SBUF) pulls the 16 selected
  k/v blocks; scores s^T = k_sel^T q (two heads packed in the 128x128 PE array
  via row groups); exp on ACT straight from PSUM; o = (e^T)^T v_sel with the
  gathered ones column yielding the softmax denominator; per-partition
  normalize; AllGather; projection from DMA-transposed slabs + bias.
"""

import os
import sys

for _p in ("/opt/trn_rl_repo", "/root/.axon_site/_ro/trn_rl_repo"):
    if os.path.isdir(_p) and _p not in sys.path:
        sys.path.insert(0, _p)

import numpy as np

import concourse.bass as bass
import concourse.bacc as bacc
import concourse.tile as tile
import concourse.mybir as mybir
from concourse.bass_utils import run_bass_kernel_spmd
from concourse.library_config import ap_gather as ap_gather_lib

# problem constants
N = 4096          # sequence length
C = 1024          # model dim
H = 16            # heads
D = 64            # head dim
BLK = 128         # block size
NB = N // BLK     # 32 blocks
TOPK = 16         # int(0.5 * NB)
NCORES = 8
HPC = H // NCORES  # 2 heads per core
SCALE = D ** -0.5  # 0.125

F32 = mybir.dt.float32
BF16 = mybir.dt.bfloat16
I16 = mybir.dt.int16
U32 = mybir.dt.uint32

_CACHE = {}


def _build():
    nc = bacc.Bacc("TRN2", target_bir_lowering=False, debug=False,
                   num_devices=NCORES)

    xT = nc.dram_tensor("xT", [C, N], F32, kind="ExternalInput")
    wqkvT = nc.dram_tensor("wqkvT", [C, 3 * 2 * D], F32, kind="ExternalInput")
    projWT = nc.dram_tensor("projWT", [C, 128], F32, kind="ExternalInput")
    projb = nc.dram_tensor("projb", [128, 1], F32, kind="ExternalInput")
    ident64 = nc.dram_tensor("ident64", [64, 64], F32, kind="ExternalInput")
    out_ext = nc.dram_tensor("out", [128, N], F32, kind="ExternalOutput")

    NAGC = 8  # AllGather token chunks
    AGT = N // NAGC
    obounce = [nc.dram_tensor(f"obounce{c}", [AGT, 2 * D], BF16) for c in range(NAGC)]
    og = [nc.dram_tensor(f"og{c}", [NCORES * AGT, 2 * D], BF16, addr_space="Shared")
          for c in range(NAGC)]

    KC = C // 128  # 8 contraction tiles

    with tile.TileContext(nc) as tc:
        nc.gpsimd.load_library(ap_gather_lib)

        with tc.tile_pool(name="persist", bufs=1) as pp:
            # ---- weights ----
            wqkv_bf = pp.tile([128, KC, 384], BF16)
            nc.gpsimd.dma_start(
                wqkv_bf[:], wqkvT.ap().rearrange("(a p) m -> p a m", p=128))
            wqk_f32 = pp.tile([128, KC, 256], F32)
            nc.sync.dma_start(
                wqk_f32[:], wqkvT.ap().rearrange("(a p) m -> p a m", p=128)[:, :, 0:256])
            projW_bf = pp.tile([128, KC, 128], BF16)
            nc.gpsimd.dma_start(
                projW_bf[:], projWT.ap().rearrange("(a p) m -> p a m", p=128))
            projb_sb = pp.tile([128, 1], F32)
            nc.sync.dma_start(projb_sb[:], projb.ap())
            id64 = pp.tile([64, 64], F32)
            nc.sync.dma_start(id64[:], ident64.ap())

            # ---- x: one f32 read; ACT casts -> xbf, DVE block sums -> xm ----
            xbf = pp.tile([128, KC, N], BF16)
            xm = pp.tile([128, KC, NB], F32)
            with tc.tile_pool(name="xload", bufs=2) as xp:
                for kc in range(KC):
                    xf = xp.tile([128, N], F32, tag="xf")
                    nc.sync.dma_start(xf[:], xT.ap()[kc * 128:(kc + 1) * 128, :])
                    nc.vector.tensor_reduce(
                        xm[:, kc, :], xf[:].rearrange("p (b t) -> p b t", t=BLK),
                        axis=mybir.AxisListType.X, op=mybir.AluOpType.add)
                    for nch in range(8):
                        nc.scalar.copy(xbf[:, kc, nch * 512:(nch + 1) * 512],
                                       xf[:, nch * 512:(nch + 1) * 512])

            # ---- QKV (bf16) ----
            qT = pp.tile([128, N], BF16)
            kT = pp.tile([128, NB, BLK], BF16)   # contiguous == [128, N]
            v0 = pp.tile([128, NB, 66], BF16)
            v1 = pp.tile([128, NB, 66], BF16)
            nc.vector.memset(v0[:, :, 64:66], 0.0)
            nc.vector.memset(v1[:, :, 64:66], 0.0)
            nc.vector.memset(v0[:, :, 64:65], 1.0)
            nc.vector.memset(v1[:, :, 64:65], 1.0)

            with tc.tile_pool(name="qkps", bufs=3, space="PSUM") as qp, \
                 tc.tile_pool(name="vps", bufs=3, space="PSUM") as vp:
                for nch in range(8):
                    for mt in (0, 1):
                        ps = qp.tile([128, 512], F32, tag="qk")
                        for kc in range(KC):
                            nc.tensor.matmul(
                                ps[:], lhsT=wqkv_bf[:, kc, mt * 128:(mt + 1) * 128],
                                rhs=xbf[:, kc, nch * 512:(nch + 1) * 512],
                                start=(kc == 0), stop=(kc == KC - 1))
                        if mt == 0:
                            nc.scalar.copy(qT[:, nch * 512:(nch + 1) * 512], ps[:])
                        else:
                            nc.scalar.copy(
                                kT[:].rearrange("p a b -> p (a b)")[:, nch * 512:(nch + 1) * 512],
                                ps[:])
                    for nt in range(4 * nch, 4 * nch + 4):
                        psv = vp.tile([128, 128], F32, tag="v")
                        for kc in range(KC):
                            nc.tensor.matmul(psv[:], lhsT=xbf[:, kc, nt * 128:(nt + 1) * 128],
                                             rhs=wqkv_bf[:, kc, 256:384],
                                             start=(kc == 0), stop=(kc == KC - 1))
                        nc.vector.tensor_copy(v0[:, nt, 0:64], psv[:, 0:64])
                        nc.vector.tensor_copy(v1[:, nt, 0:64], psv[:, 64:128])

            # ---- block-mean similarity + top-k selection (f32) ----
            kidx = pp.tile([128, NB], I16)
            vidx0 = pp.tile([128, NB], I16)
            vidx1 = pp.tile([128, NB], I16)
            with tc.tile_pool(name="selps", bufs=2, space="PSUM") as sp, \
                 tc.tile_pool(name="selsb", bufs=2) as sb:
                qm_ps = sp.tile([128, NB], F32, tag="qkm")
                km_ps = sp.tile([128, NB], F32, tag="qkm")
                for kc in range(KC):
                    nc.tensor.matmul(qm_ps[:], lhsT=wqk_f32[:, kc, 0:128],
                                     rhs=xm[:, kc, :], start=(kc == 0), stop=(kc == KC - 1))
                for kc in range(KC):
                    nc.tensor.matmul(km_ps[:], lhsT=wqk_f32[:, kc, 128:256],
                                     rhs=xm[:, kc, :], start=(kc == 0), stop=(kc == KC - 1))
                qm_sb = sb.tile([128, NB], F32, tag="qm")
                km_sb = sb.tile([128, NB], F32, tag="km")
                nc.scalar.copy(qm_sb[:], qm_ps[:])
                nc.scalar.copy(km_sb[:], km_ps[:])

                sim_ps = sp.tile([64, NB], F32, tag="sim")
                for h in range(HPC):
                    nc.tensor.matmul(sim_ps[h * 32:(h + 1) * 32, :],
                                     lhsT=qm_sb[h * 64:(h + 1) * 64, :],
                                     rhs=km_sb[h * 64:(h + 1) * 64, :],
                                     start=True, stop=True)
                sim2 = sb.tile([64, NB], F32, tag="sim2")
                nc.vector.tensor_copy(sim2[:], sim_ps[:])

                vals0 = sb.tile([64, 8], F32, tag="v0")
                idx0 = sb.tile([64, 8], U32, tag="i0")
                pun = sb.tile([64, NB], F32, tag="pun")
                vals1 = sb.tile([64, 8], F32, tag="v1")
                idx1 = sb.tile([64, 8], U32, tag="i1")
                nc.vector.max(vals0[:], sim2[:])
                nc.vector.max_index(idx0[:], vals0[:], sim2[:])
                nc.vector.match_replace(out=pun[:], in_to_replace=vals0[:],
                                        in_values=sim2[:], imm_value=-1e30)
                nc.vector.max(vals1[:], pun[:])
                nc.vector.max_index(idx1[:], vals1[:], pun[:])

                idxf = sb.tile([64, TOPK], F32, tag="idxf")
                nc.vector.tensor_copy(idxf[:, 0:8], idx0[:])
                nc.vector.tensor_copy(idxf[:, 8:16], idx1[:])

                selT_ps = sp.tile([TOPK, 64], F32, tag="selT")
                nc.tensor.transpose(selT_ps[:], idxf[:], id64[:])
                selT = sb.tile([TOPK, 64], I16, tag="selTsb")
                nc.vector.tensor_copy(selT[:], selT_ps[:])

                # replicate selection into per-16-partition index tiles:
                #  kidx: partitions 0-63 <- h0 cols, 64-127 <- h1 cols
                #  vidx0 <- h0 cols everywhere, vidx1 <- h1 cols everywhere
                for g in range(8):
                    half = selT[:, 0:32] if g < 4 else selT[:, 32:64]
                    nc.sync.dma_start(kidx[16 * g:16 * (g + 1), :], half)
                    nc.sync.dma_start(vidx0[16 * g:16 * (g + 1), :], selT[:, 0:32])
                    nc.sync.dma_start(vidx1[16 * g:16 * (g + 1), :], selT[:, 32:64])

            # ---- main block-sparse attention loop + chunked AG + projection ----
            out_sb = pp.tile([128, N], F32)
            with tc.tile_pool(name="gather", bufs=3) as gp, \
                 tc.tile_pool(name="escore", bufs=10) as ep, \
                 tc.tile_pool(name="sps", bufs=4, space="PSUM") as spp, \
                 tc.tile_pool(name="ops", bufs=2, space="PSUM") as opp, \
                 tc.tile_pool(name="prj", bufs=2, space="PSUM") as jp, \
                 tc.tile_pool(name="otp", bufs=3) as otp, \
                 tc.tile_pool(name="osb", bufs=3) as ob:
                def _emit_proj(c):
                    pj = jp.tile([128, AGT], F32, tag="pj", name=f"pj_{c}")
                    for r in range(NCORES):
                        ot = otp.tile([128, AGT], BF16, tag="ot", name=f"ot_{c}_{r}")
                        nc.sync.dma_start_transpose(
                            ot[:], og[c].ap()[r * AGT:(r + 1) * AGT, :])
                        nc.tensor.matmul(pj[:], lhsT=projW_bf[:, r, :], rhs=ot[:],
                                         start=(r == 0), stop=(r == NCORES - 1))
                    nc.scalar.activation(out_sb[:, c * AGT:(c + 1) * AGT], pj[:],
                                         mybir.ActivationFunctionType.Identity,
                                         bias=projb_sb[:])
                    nc.sync.dma_start(out_ext.ap()[:, c * AGT:(c + 1) * AGT],
                                      out_sb[:, c * AGT:(c + 1) * AGT])

                for qb in range(NB):
                    kg = gp.tile([128, TOPK, BLK], BF16, tag="kg")
                    nc.gpsimd.ap_gather(kg[:], kT[:], kidx[:, qb:qb + 1],
                                        channels=128, num_elems=NB, d=BLK, num_idxs=TOPK)
                    vg0 = gp.tile([128, TOPK, 66], BF16, tag="vg0")
                    nc.gpsimd.ap_gather(vg0[:], v0[:], vidx0[:, qb:qb + 1],
                                        channels=128, num_elems=NB, d=66, num_idxs=TOPK)
                    vg1 = gp.tile([128, TOPK, 66], BF16, tag="vg1")
                    nc.gpsimd.ap_gather(vg1[:], v1[:], vidx1[:, qb:qb + 1],
                                        channels=128, num_elems=NB, d=66, num_idxs=TOPK)

                    qcol = slice(qb * BLK, (qb + 1) * BLK)
                    etiles = [[], []]
                    for quarter in range(4):
                        s0 = spp.tile([128, 512], F32, tag="s", name=f"s0_{qb}_{quarter}")
                        s1 = spp.tile([128, 512], F32, tag="s", name=f"s1_{qb}_{quarter}")
                        for jj in range(4):
                            j = quarter * 4 + jj
                            nc.tensor.matmul(s0[:, jj * 128:(jj + 1) * 128],
                                             lhsT=kg[0:64, j, :], rhs=qT[0:64, qcol],
                                             start=True, stop=True)
                            nc.tensor.matmul(s1[:, jj * 128:(jj + 1) * 128],
                                             lhsT=kg[64:128, j, :], rhs=qT[64:128, qcol],
                                             start=True, stop=True)
                        e0 = ep.tile([128, 512], BF16, tag="e", name=f"e0_{qb}_{quarter}")
                        e1 = ep.tile([128, 512], BF16, tag="e", name=f"e1_{qb}_{quarter}")
                        nc.scalar.activation(e0[:], s0[:],
                                             mybir.ActivationFunctionType.Exp, scale=SCALE)
                        nc.scalar.activation(e1[:], s1[:],
                                             mybir.ActivationFunctionType.Exp, scale=SCALE)
                        etiles[0].append(e0)
                        etiles[1].append(e1)

                    onorm = ob.tile([128, 2 * D], BF16, tag="onorm")
                    for h in range(HPC):
                        vg = vg0 if h == 0 else vg1
                        o_ps = opp.tile([128, D + 1], F32, tag="o")
                        for j in range(TOPK):
                            nc.tensor.matmul(o_ps[:],
                                             lhsT=etiles[h][j // 4][:, (j % 4) * 128:(j % 4 + 1) * 128],
                                             rhs=vg[:, j, 0:D + 1],
                                             start=(j == 0), stop=(j == TOPK - 1))
                        rec = ob.tile([128, 1], F32, tag="rec")
                        nc.vector.reciprocal(rec[:], o_ps[:, D:D + 1])
                        nc.vector.tensor_scalar(onorm[:, h * D:(h + 1) * D],
                                                o_ps[:, 0:D], rec[:], None,
                                                op0=mybir.AluOpType.mult)
                    ch = qb // (NB // NAGC)
                    qo = qb % (NB // NAGC)
                    nc.gpsimd.dma_start(
                        obounce[ch].ap()[qo * BLK:(qo + 1) * BLK, :], onorm[:])
                    if qo == NB // NAGC - 1:
                        nc.gpsimd.collective_compute(
                            "AllGather", mybir.AluOpType.bypass,
                            replica_groups=[list(range(NCORES))],
                            ins=[obounce[ch].ap().opt()], outs=[og[ch].ap().opt()])
                    # projection for a chunk whose AllGather fired 2 chunks ago
                    if qo == NB // NAGC - 1 and ch >= 2:
                        _emit_proj(ch - 2)

                for c in (NAGC - 2, NAGC - 1):
                    _emit_proj(c)

    nc.compile()
    return nc


def _prep_inputs(x, qkv_w, proj_w, proj_b):
    x = np.asarray(x, dtype=np.float32)
    qkv_w = np.asarray(qkv_w, dtype=np.float32)
    proj_w = np.asarray(proj_w, dtype=np.float32)
    proj_b = np.asarray(proj_b, dtype=np.float32)

    xT = np.ascontiguousarray(x[0].T)                      # [C, N]
    ident64 = np.eye(64, dtype=np.float32)
    in_maps = []
    for i in range(NCORES):
        h0 = HPC * i
        rows = []
        for part in range(3):                              # q, k, v row groups
            base = part * C + h0 * D
            rows.append(qkv_w[base:base + HPC * D, :])
        wqkv = np.concatenate(rows, axis=0)                # [384, C]
        in_maps.append({
            "xT": xT,
            "wqkvT": np.ascontiguousarray(wqkv.T),         # [C, 384]
            "projWT": np.ascontiguousarray(proj_w[i * 128:(i + 1) * 128, :].T),
            "projb": np.ascontiguousarray(proj_b[i * 128:(i + 1) * 128]).reshape(128, 1),
            "ident64": ident64,
        })
    return in_maps


def kernel(x, qkv_w, proj_w, proj_b, _trace=False):
    if "nc" not in _CACHE:
        _CACHE["nc"] = _build()
    nc = _CACHE["nc"]
    in_maps = _prep_inputs(x, qkv_w, proj_w, proj_b)
    res = run_bass_kernel_spmd(nc, in_maps, core_ids=list(range(NCORES)),
                               trace=_trace)
    outT = np.concatenate([res.results[i]["out"] for i in range(NCORES)], axis=0)
    out = np.ascontiguousarray(outT.T).reshape(1, N, C).astype(np.float32)
    if _trace:
        _CACHE["last_exec_time_ns"] = res.exec_time_ns
        _CACHE["last_results"] = res
    return out


# revision 13
# speedup vs baseline: 1.1773x; 1.1773x over previous
"""Block-sparse attention (SageAttention-style mean-similarity top-k) on 8 TRN2 NeuronCores.

Sharding: 16 heads tensor-parallel across 8 cores (2 heads/core).
  - qkv weight column-sharded per core (its 2 heads' q/k/v rows, pre-transposed on host)
  - attention (block selection + block-sparse softmax-V) fully local per head
  - attention outputs AllGathered (token-major bf16), proj weight row-sharded:
    each core computes a 128-column slice of the output; host concatenates.

Per-core device pipeline (bf16 compute, f32 selection):
  x^T f32 -> block sums (DVE) -> qm/km/sim (f32 PE) -> top-16 via max8/max_index
  qkv matmuls (bf16 PE), k kept d-major, v token-major with a ones column
  per query block: ap_gather (GPSIMD ucode, SBUF->SBUF) pulls the 16 selected
  k/v blocks; scores s^T = k_sel^T q (two heads packed in the 128x128 PE array
  via row groups); exp on ACT straight from PSUM; o = (e^T)^T v_sel with the
  gathered ones column yielding the softmax denominator; per-partition
  normalize; AllGather; projection from DMA-transposed slabs + bias.
"""

import os
import sys

for _p in ("/opt/trn_rl_repo", "/root/.axon_site/_ro/trn_rl_repo"):
    if os.path.isdir(_p) and _p not in sys.path:
        sys.path.insert(0, _p)

import numpy as np

import concourse.bass as bass
import concourse.bacc as bacc
import concourse.tile as tile
import concourse.mybir as mybir
from concourse.bass_utils import run_bass_kernel_spmd
from concourse.library_config import ap_gather as ap_gather_lib

# problem constants
N = 4096          # sequence length
C = 1024          # model dim
H = 16            # heads
D = 64            # head dim
BLK = 128         # block size
NB = N // BLK     # 32 blocks
TOPK = 16         # int(0.5 * NB)
NCORES = 8
HPC = H // NCORES  # 2 heads per core
SCALE = D ** -0.5  # 0.125

F32 = mybir.dt.float32
BF16 = mybir.dt.bfloat16
I16 = mybir.dt.int16
U32 = mybir.dt.uint32

_CACHE = {}


def _build():
    nc = bacc.Bacc("TRN2", target_bir_lowering=False, debug=False,
                   num_devices=NCORES)

    xT = nc.dram_tensor("xT", [C, N], F32, kind="ExternalInput")
    wqkvT = nc.dram_tensor("wqkvT", [C, 3 * 2 * D], F32, kind="ExternalInput")
    projWT = nc.dram_tensor("projWT", [C, 128], F32, kind="ExternalInput")
    projb = nc.dram_tensor("projb", [128, 1], F32, kind="ExternalInput")
    ident64 = nc.dram_tensor("ident64", [64, 64], F32, kind="ExternalInput")
    out_ext = nc.dram_tensor("out", [128, N], F32, kind="ExternalOutput")

    NAGC = 2  # AllGather token chunks
    AGT = N // NAGC
    obounce = [nc.dram_tensor(f"obounce{c}", [AGT, 2 * D], BF16) for c in range(NAGC)]
    og = [nc.dram_tensor(f"og{c}", [NCORES * AGT, 2 * D], BF16, addr_space="Shared")
          for c in range(NAGC)]

    KC = C // 128  # 8 contraction tiles

    with tile.TileContext(nc) as tc:
        nc.gpsimd.load_library(ap_gather_lib)

        with tc.tile_pool(name="persist", bufs=1) as pp:
            # ---- weights ----
            wqkv_bf = pp.tile([128, KC, 384], BF16)
            nc.gpsimd.dma_start(
                wqkv_bf[:], wqkvT.ap().rearrange("(a p) m -> p a m", p=128))
            wqk_f32 = pp.tile([128, KC, 256], F32)
            nc.sync.dma_start(
                wqk_f32[:], wqkvT.ap().rearrange("(a p) m -> p a m", p=128)[:, :, 0:256])
            projW_bf = pp.tile([128, KC, 128], BF16)
            nc.gpsimd.dma_start(
                projW_bf[:], projWT.ap().rearrange("(a p) m -> p a m", p=128))
            projb_sb = pp.tile([128, 1], F32)
            nc.sync.dma_start(projb_sb[:], projb.ap())
            id64 = pp.tile([64, 64], F32)
            nc.sync.dma_start(id64[:], ident64.ap())

            # ---- x: bf16 copy (cast during DMA) + f32 block sums ----
            xbf = pp.tile([128, KC, N], BF16)
            for nch in range(8):
                nc.gpsimd.dma_start(
                    xbf[:, :, nch * 512:(nch + 1) * 512],
                    xT.ap().rearrange("(a p) m -> p a m", p=128)[:, :, nch * 512:(nch + 1) * 512])

            xm = pp.tile([128, KC, NB], F32)
            with tc.tile_pool(name="xload", bufs=2) as xp:
                for kc in range(KC):
                    xf = xp.tile([128, N], F32, tag="xf")
                    nc.sync.dma_start(xf[:], xT.ap()[kc * 128:(kc + 1) * 128, :])
                    nc.vector.tensor_reduce(
                        xm[:, kc, :], xf[:].rearrange("p (b t) -> p b t", t=BLK),
                        axis=mybir.AxisListType.X, op=mybir.AluOpType.add)

            # ---- QKV (bf16) ----
            qT = pp.tile([128, N], BF16)
            kT = pp.tile([128, NB, BLK], BF16)   # contiguous == [128, N]
            v0 = pp.tile([128, NB, 66], BF16)
            v1 = pp.tile([128, NB, 66], BF16)
            nc.vector.memset(v0[:, :, 64:66], 0.0)
            nc.vector.memset(v1[:, :, 64:66], 0.0)
            nc.vector.memset(v0[:, :, 64:65], 1.0)
            nc.vector.memset(v1[:, :, 64:65], 1.0)

            with tc.tile_pool(name="qkps", bufs=3, space="PSUM") as qp:
                for mt, dst in ((0, qT), (1, None)):
                    for nch in range(8):
                        ps = qp.tile([128, 512], F32, tag="qk")
                        for kc in range(KC):
                            nc.tensor.matmul(
                                ps[:], lhsT=wqkv_bf[:, kc, mt * 128:(mt + 1) * 128],
                                rhs=xbf[:, kc, nch * 512:(nch + 1) * 512],
                                start=(kc == 0), stop=(kc == KC - 1))
                        if mt == 0:
                            nc.scalar.copy(qT[:, nch * 512:(nch + 1) * 512], ps[:])
                        else:
                            nc.scalar.copy(
                                kT[:].rearrange("p a b -> p (a b)")[:, nch * 512:(nch + 1) * 512],
                                ps[:])

            with tc.tile_pool(name="vps", bufs=3, space="PSUM") as vp:
                for nt in range(NB):
                    ps = vp.tile([128, 128], F32, tag="v")
                    for kc in range(KC):
                        nc.tensor.matmul(ps[:], lhsT=xbf[:, kc, nt * 128:(nt + 1) * 128],
                                         rhs=wqkv_bf[:, kc, 256:384],
                                         start=(kc == 0), stop=(kc == KC - 1))
                    nc.vector.tensor_copy(v0[:, nt, 0:64], ps[:, 0:64])
                    nc.vector.tensor_copy(v1[:, nt, 0:64], ps[:, 64:128])

            # ---- block-mean similarity + top-k selection (f32) ----
            kidx = pp.tile([128, NB], I16)
            vidx0 = pp.tile([128, NB], I16)
            vidx1 = pp.tile([128, NB], I16)
            with tc.tile_pool(name="selps", bufs=2, space="PSUM") as sp, \
                 tc.tile_pool(name="selsb", bufs=2) as sb:
                qm_ps = sp.tile([128, NB], F32, tag="qkm")
                km_ps = sp.tile([128, NB], F32, tag="qkm")
                for kc in range(KC):
                    nc.tensor.matmul(qm_ps[:], lhsT=wqk_f32[:, kc, 0:128],
                                     rhs=xm[:, kc, :], start=(kc == 0), stop=(kc == KC - 1))
                for kc in range(KC):
                    nc.tensor.matmul(km_ps[:], lhsT=wqk_f32[:, kc, 128:256],
                                     rhs=xm[:, kc, :], start=(kc == 0), stop=(kc == KC - 1))
                qm_sb = sb.tile([128, NB], F32, tag="qm")
                km_sb = sb.tile([128, NB], F32, tag="km")
                nc.scalar.copy(qm_sb[:], qm_ps[:])
                nc.scalar.copy(km_sb[:], km_ps[:])

                sim_ps = sp.tile([64, NB], F32, tag="sim")
                for h in range(HPC):
                    nc.tensor.matmul(sim_ps[h * 32:(h + 1) * 32, :],
                                     lhsT=qm_sb[h * 64:(h + 1) * 64, :],
                                     rhs=km_sb[h * 64:(h + 1) * 64, :],
                                     start=True, stop=True)
                sim2 = sb.tile([64, NB], F32, tag="sim2")
                nc.vector.tensor_copy(sim2[:], sim_ps[:])

                vals0 = sb.tile([64, 8], F32, tag="v0")
                idx0 = sb.tile([64, 8], U32, tag="i0")
                pun = sb.tile([64, NB], F32, tag="pun")
                vals1 = sb.tile([64, 8], F32, tag="v1")
                idx1 = sb.tile([64, 8], U32, tag="i1")
                nc.vector.max(vals0[:], sim2[:])
                nc.vector.max_index(idx0[:], vals0[:], sim2[:])
                nc.vector.match_replace(out=pun[:], in_to_replace=vals0[:],
                                        in_values=sim2[:], imm_value=-1e30)
                nc.vector.max(vals1[:], pun[:])
                nc.vector.max_index(idx1[:], vals1[:], pun[:])

                idxf = sb.tile([64, TOPK], F32, tag="idxf")
                nc.vector.tensor_copy(idxf[:, 0:8], idx0[:])
                nc.vector.tensor_copy(idxf[:, 8:16], idx1[:])

                selT_ps = sp.tile([TOPK, 64], F32, tag="selT")
                nc.tensor.transpose(selT_ps[:], idxf[:], id64[:])
                selT = sb.tile([TOPK, 64], I16, tag="selTsb")
                nc.vector.tensor_copy(selT[:], selT_ps[:])

                # replicate selection into per-16-partition index tiles:
                #  kidx: partitions 0-63 <- h0 cols, 64-127 <- h1 cols
                #  vidx0 <- h0 cols everywhere, vidx1 <- h1 cols everywhere
                for g in range(8):
                    half = selT[:, 0:32] if g < 4 else selT[:, 32:64]
                    nc.sync.dma_start(kidx[16 * g:16 * (g + 1), :], half)
                    nc.sync.dma_start(vidx0[16 * g:16 * (g + 1), :], selT[:, 0:32])
                    nc.sync.dma_start(vidx1[16 * g:16 * (g + 1), :], selT[:, 32:64])

            # ---- main block-sparse attention loop ----
            out_sb = pp.tile([128, N], F32)
            with tc.tile_pool(name="gather", bufs=3) as gp, \
                 tc.tile_pool(name="escore", bufs=6) as ep, \
                 tc.tile_pool(name="sps", bufs=3, space="PSUM") as spp, \
                 tc.tile_pool(name="ops", bufs=2, space="PSUM") as opp, \
                 tc.tile_pool(name="otp", bufs=2) as otp, \
                 tc.tile_pool(name="osb", bufs=3) as ob:

                def _emit_proj(c):
                    for half in range(2):
                        pj = spp.tile([128, 1024], F32, tag="s", name=f"pj_{c}_{half}")
                        for r in range(NCORES):
                            ot = otp.tile([128, 1024], BF16, tag=f"ot{half}",
                                          name=f"ot_{c}_{half}_{r}")
                            nc.sync.dma_start_transpose(
                                ot[:], og[c].ap()[r * AGT + half * 1024:r * AGT + (half + 1) * 1024, :])
                            for s2 in range(2):
                                nc.tensor.matmul(pj[:, s2 * 512:(s2 + 1) * 512],
                                                 lhsT=projW_bf[:, r, :],
                                                 rhs=ot[:, s2 * 512:(s2 + 1) * 512],
                                                 start=(r == 0), stop=(r == NCORES - 1))
                        base = c * AGT + half * 1024
                        for s2 in range(2):
                            nc.scalar.activation(
                                out_sb[:, base + s2 * 512:base + (s2 + 1) * 512],
                                pj[:, s2 * 512:(s2 + 1) * 512],
                                mybir.ActivationFunctionType.Identity,
                                bias=projb_sb[:])
                        nc.sync.dma_start(out_ext.ap()[:, base:base + 1024],
                                          out_sb[:, base:base + 1024])

                for qb in range(NB):
                    kg = gp.tile([128, TOPK, BLK], BF16, tag="kg")
                    nc.gpsimd.ap_gather(kg[:], kT[:], kidx[:, qb:qb + 1],
                                        channels=128, num_elems=NB, d=BLK, num_idxs=TOPK)
                    vg0 = gp.tile([128, TOPK, 66], BF16, tag="vg0")
                    nc.gpsimd.ap_gather(vg0[:], v0[:], vidx0[:, qb:qb + 1],
                                        channels=128, num_elems=NB, d=66, num_idxs=TOPK)
                    vg1 = gp.tile([128, TOPK, 66], BF16, tag="vg1")
                    nc.gpsimd.ap_gather(vg1[:], v1[:], vidx1[:, qb:qb + 1],
                                        channels=128, num_elems=NB, d=66, num_idxs=TOPK)

                    qcol = slice(qb * BLK, (qb + 1) * BLK)
                    etiles = [[None, None], [None, None]]
                    for half in range(2):
                        s0 = spp.tile([128, 1024], F32, tag="s")
                        s1 = spp.tile([128, 1024], F32, tag="s")
                        for jj in range(8):
                            j = half * 8 + jj
                            nc.tensor.matmul(s0[:, jj * 128:(jj + 1) * 128],
                                             lhsT=kg[0:64, j, :], rhs=qT[0:64, qcol],
                                             start=True, stop=True)
                            nc.tensor.matmul(s1[:, jj * 128:(jj + 1) * 128],
                                             lhsT=kg[64:128, j, :], rhs=qT[64:128, qcol],
                                             start=True, stop=True)
                        e0 = ep.tile([128, 1024], BF16, tag="e")
                        e1 = ep.tile([128, 1024], BF16, tag="e")
                        nc.scalar.activation(e0[:], s0[:],
                                             mybir.ActivationFunctionType.Exp, scale=SCALE)
                        nc.scalar.activation(e1[:], s1[:],
                                             mybir.ActivationFunctionType.Exp, scale=SCALE)
                        etiles[0][half] = e0
                        etiles[1][half] = e1

                    onorm = ob.tile([128, 2 * D], BF16, tag="onorm")
                    for h in range(HPC):
                        vg = vg0 if h == 0 else vg1
                        o_ps = opp.tile([128, D + 1], F32, tag="o")
                        for j in range(TOPK):
                            nc.tensor.matmul(o_ps[:],
                                             lhsT=etiles[h][j // 8][:, (j % 8) * 128:(j % 8 + 1) * 128],
                                             rhs=vg[:, j, 0:D + 1],
                                             start=(j == 0), stop=(j == TOPK - 1))
                        rec = ob.tile([128, 1], F32, tag="rec")
                        nc.vector.reciprocal(rec[:], o_ps[:, D:D + 1])
                        nc.vector.tensor_scalar(onorm[:, h * D:(h + 1) * D],
                                                o_ps[:, 0:D], rec[:], None,
                                                op0=mybir.AluOpType.mult)
                    ch = qb // (NB // NAGC)
                    nc.sync.dma_start(
                        obounce[ch].ap()[(qb % (NB // NAGC)) * BLK:(qb % (NB // NAGC) + 1) * BLK, :],
                        onorm[:])
                    if qb % (NB // NAGC) == NB // NAGC - 1:
                        nc.gpsimd.collective_compute(
                            "AllGather", mybir.AluOpType.bypass,
                            replica_groups=[list(range(NCORES))],
                            ins=[obounce[ch].ap().opt()], outs=[og[ch].ap().opt()])
                    if qb == 24:
                        _emit_proj(0)

                _emit_proj(1)

    nc.compile()
    return nc


def _prep_inputs(x, qkv_w, proj_w, proj_b):
    x = np.asarray(x, dtype=np.float32)
    qkv_w = np.asarray(qkv_w, dtype=np.float32)
    proj_w = np.asarray(proj_w, dtype=np.float32)
    proj_b = np.asarray(proj_b, dtype=np.float32)

    xT = np.ascontiguousarray(x[0].T)                      # [C, N]
    ident64 = np.eye(64, dtype=np.float32)
    in_maps = []
    for i in range(NCORES):
        h0 = HPC * i
        rows = []
        for part in range(3):                              # q, k, v row groups
            base = part * C + h0 * D
            rows.append(qkv_w[base:base + HPC * D, :])
        wqkv = np.concatenate(rows, axis=0)                # [384, C]
        in_maps.append({
            "xT": xT,
            "wqkvT": np.ascontiguousarray(wqkv.T),         # [C, 384]
            "projWT": np.ascontiguousarray(proj_w[i * 128:(i + 1) * 128, :].T),
            "projb": np.ascontiguousarray(proj_b[i * 128:(i + 1) * 128]).reshape(128, 1),
            "ident64": ident64,
        })
    return in_maps


def kernel(x, qkv_w, proj_w, proj_b, _trace=False):
    if "nc" not in _CACHE:
        _CACHE["nc"] = _build()
    nc = _CACHE["nc"]
    in_maps = _prep_inputs(x, qkv_w, proj_w, proj_b)
    res = run_bass_kernel_spmd(nc, in_maps, core_ids=list(range(NCORES)),
                               trace=_trace)
    outT = np.concatenate([res.results[i]["out"] for i in range(NCORES)], axis=0)
    out = np.ascontiguousarray(outT.T).reshape(1, N, C).astype(np.float32)
    if _trace:
        _CACHE["last_exec_time_ns"] = res.exec_time_ns
        _CACHE["last_results"] = res
    return out


# revision 14
# speedup vs baseline: 1.3113x; 1.1138x over previous
"""Block-sparse attention (SageAttention-style mean-similarity top-k) on 8 TRN2 NeuronCores.

Sharding: 16 heads tensor-parallel across 8 cores (2 heads/core).
  - qkv weight column-sharded per core (its 2 heads' q/k/v rows, pre-transposed on host)
  - block selection + block-sparse attention fully local per head
  - proj weight row-sharded: each core computes the full-shape PARTIAL product
    o_local @ projW[:, c_slice].T (+ bias on core 0 only); the host unshard step
    sums the 8 partials (the row-parallel reduction).

Per-core device pipeline (bf16 compute, f32 selection):
  x^T f32 -> block sums (DVE) -> qm/km/sim (f32 PE) -> top-16 via max8/max_index
  qkv matmuls (bf16 PE), k kept d-major, v token-major with a ones column
  per query block: ap_gather (GPSIMD ucode, SBUF->SBUF) pulls the 16 selected
  k/v blocks; scores s^T = k_sel^T q (two heads packed in the 128x128 PE array
  via row groups); exp on ACT straight from PSUM; o = (e^T)^T v_sel with the
  gathered ones column yielding the softmax denominator; per-partition
  normalize; chunk-wise DMA-transpose + projection partials streamed out.
"""

import os
import sys

for _p in ("/opt/trn_rl_repo", "/root/.axon_site/_ro/trn_rl_repo"):
    if os.path.isdir(_p) and _p not in sys.path:
        sys.path.insert(0, _p)

import numpy as np

import concourse.bass as bass
import concourse.bacc as bacc
import concourse.tile as tile
import concourse.mybir as mybir
from concourse.bass_utils import run_bass_kernel_spmd
from concourse.library_config import ap_gather as ap_gather_lib

# problem constants
N = 4096          # sequence length
C = 1024          # model dim
H = 16            # heads
D = 64            # head dim
BLK = 128         # block size
NB = N // BLK     # 32 blocks
TOPK = 16         # int(0.5 * NB)
NCORES = 8
HPC = H // NCORES  # 2 heads per core
SCALE = D ** -0.5  # 0.125

F32 = mybir.dt.float32
BF16 = mybir.dt.bfloat16
I16 = mybir.dt.int16
U32 = mybir.dt.uint32

_CACHE = {}


def _build():
    nc = bacc.Bacc("TRN2", target_bir_lowering=False, debug=False,
                   num_devices=NCORES)

    KC = C // 128  # 8 contraction tiles

    xT = nc.dram_tensor("xT", [C, N], F32, kind="ExternalInput")
    wqkvT = nc.dram_tensor("wqkvT", [C, 3 * 2 * D], F32, kind="ExternalInput")
    projWT = nc.dram_tensor("projWT", [2 * D, C], F32, kind="ExternalInput")
    projb = nc.dram_tensor("projb", [128, KC], F32, kind="ExternalInput")
    ident64 = nc.dram_tensor("ident64", [64, 64], F32, kind="ExternalInput")
    out_ext = nc.dram_tensor("out", [C, N], F32, kind="ExternalOutput")

    obounce = nc.dram_tensor("obounce", [N, 2 * D], BF16)

    with tile.TileContext(nc) as tc:
        nc.gpsimd.load_library(ap_gather_lib)

        with tc.tile_pool(name="persist", bufs=1) as pp:
            # ---- weights ----
            wqkv_bf = pp.tile([128, KC, 384], BF16)
            nc.gpsimd.dma_start(
                wqkv_bf[:], wqkvT.ap().rearrange("(a p) m -> p a m", p=128))
            wqk_f32 = pp.tile([128, KC, 256], F32)
            nc.sync.dma_start(
                wqk_f32[:], wqkvT.ap().rearrange("(a p) m -> p a m", p=128)[:, :, 0:256])
            projW_bf = pp.tile([128, C], BF16)          # [c_local, j]
            nc.gpsimd.dma_start(projW_bf[:], projWT.ap())
            projb_sb = pp.tile([128, KC], F32)          # bias for j-tile m in col m
            nc.sync.dma_start(projb_sb[:], projb.ap())
            id64 = pp.tile([64, 64], F32)
            nc.sync.dma_start(id64[:], ident64.ap())

            # ---- x: bf16 copy (cast during DMA) + f32 block sums ----
            xbf = pp.tile([128, KC, N], BF16)
            for nch in range(8):
                nc.gpsimd.dma_start(
                    xbf[:, :, nch * 512:(nch + 1) * 512],
                    xT.ap().rearrange("(a p) m -> p a m", p=128)[:, :, nch * 512:(nch + 1) * 512])

            xm = pp.tile([128, KC, NB], F32)
            with tc.tile_pool(name="xload", bufs=2) as xp:
                for kc in range(KC):
                    xf = xp.tile([128, N], F32, tag="xf")
                    nc.sync.dma_start(xf[:], xT.ap()[kc * 128:(kc + 1) * 128, :])
                    nc.vector.tensor_reduce(
                        xm[:, kc, :], xf[:].rearrange("p (b t) -> p b t", t=BLK),
                        axis=mybir.AxisListType.X, op=mybir.AluOpType.add)

            # ---- QKV (bf16) ----
            qT = pp.tile([128, N], BF16)
            kT = pp.tile([128, NB, BLK], BF16)   # contiguous == [128, N]
            v0 = pp.tile([128, NB, 66], BF16)
            v1 = pp.tile([128, NB, 66], BF16)
            nc.vector.memset(v0[:, :, 64:66], 0.0)
            nc.vector.memset(v1[:, :, 64:66], 0.0)
            nc.vector.memset(v0[:, :, 64:65], 1.0)
            nc.vector.memset(v1[:, :, 64:65], 1.0)

            with tc.tile_pool(name="qkps", bufs=3, space="PSUM") as qp:
                for mt in (0, 1):
                    for nch in range(8):
                        ps = qp.tile([128, 512], F32, tag="qk")
                        for kc in range(KC):
                            nc.tensor.matmul(
                                ps[:], lhsT=wqkv_bf[:, kc, mt * 128:(mt + 1) * 128],
                                rhs=xbf[:, kc, nch * 512:(nch + 1) * 512],
                                start=(kc == 0), stop=(kc == KC - 1))
                        if mt == 0:
                            nc.scalar.copy(qT[:, nch * 512:(nch + 1) * 512], ps[:])
                        else:
                            nc.scalar.copy(
                                kT[:].rearrange("p a b -> p (a b)")[:, nch * 512:(nch + 1) * 512],
                                ps[:])

            with tc.tile_pool(name="vps", bufs=3, space="PSUM") as vp:
                for nt in range(NB):
                    ps = vp.tile([128, 128], F32, tag="v")
                    for kc in range(KC):
                        nc.tensor.matmul(ps[:], lhsT=xbf[:, kc, nt * 128:(nt + 1) * 128],
                                         rhs=wqkv_bf[:, kc, 256:384],
                                         start=(kc == 0), stop=(kc == KC - 1))
                    nc.vector.tensor_copy(v0[:, nt, 0:64], ps[:, 0:64])
                    nc.vector.tensor_copy(v1[:, nt, 0:64], ps[:, 64:128])

            # ---- block-mean similarity + top-k selection (f32) ----
            kidx = pp.tile([128, NB], I16)
            vidx0 = pp.tile([128, NB], I16)
            vidx1 = pp.tile([128, NB], I16)
            with tc.tile_pool(name="selps", bufs=2, space="PSUM") as sp, \
                 tc.tile_pool(name="selsb", bufs=2) as sb:
                qm_ps = sp.tile([128, NB], F32, tag="qkm")
                km_ps = sp.tile([128, NB], F32, tag="qkm")
                for kc in range(KC):
                    nc.tensor.matmul(qm_ps[:], lhsT=wqk_f32[:, kc, 0:128],
                                     rhs=xm[:, kc, :], start=(kc == 0), stop=(kc == KC - 1))
                for kc in range(KC):
                    nc.tensor.matmul(km_ps[:], lhsT=wqk_f32[:, kc, 128:256],
                                     rhs=xm[:, kc, :], start=(kc == 0), stop=(kc == KC - 1))
                qm_sb = sb.tile([128, NB], F32, tag="qm")
                km_sb = sb.tile([128, NB], F32, tag="km")
                nc.scalar.copy(qm_sb[:], qm_ps[:])
                nc.scalar.copy(km_sb[:], km_ps[:])

                sim_ps = sp.tile([64, NB], F32, tag="sim")
                for h in range(HPC):
                    nc.tensor.matmul(sim_ps[h * 32:(h + 1) * 32, :],
                                     lhsT=qm_sb[h * 64:(h + 1) * 64, :],
                                     rhs=km_sb[h * 64:(h + 1) * 64, :],
                                     start=True, stop=True)
                sim2 = sb.tile([64, NB], F32, tag="sim2")
                nc.vector.tensor_copy(sim2[:], sim_ps[:])

                vals0 = sb.tile([64, 8], F32, tag="v0")
                idx0 = sb.tile([64, 8], U32, tag="i0")
                pun = sb.tile([64, NB], F32, tag="pun")
                vals1 = sb.tile([64, 8], F32, tag="v1")
                idx1 = sb.tile([64, 8], U32, tag="i1")
                nc.vector.max(vals0[:], sim2[:])
                nc.vector.max_index(idx0[:], vals0[:], sim2[:])
                nc.vector.match_replace(out=pun[:], in_to_replace=vals0[:],
                                        in_values=sim2[:], imm_value=-1e30)
                nc.vector.max(vals1[:], pun[:])
                nc.vector.max_index(idx1[:], vals1[:], pun[:])

                idxf = sb.tile([64, TOPK], F32, tag="idxf")
                nc.vector.tensor_copy(idxf[:, 0:8], idx0[:])
                nc.vector.tensor_copy(idxf[:, 8:16], idx1[:])

                selT_ps = sp.tile([TOPK, 64], F32, tag="selT")
                nc.tensor.transpose(selT_ps[:], idxf[:], id64[:])
                selT = sb.tile([TOPK, 64], I16, tag="selTsb")
                nc.vector.tensor_copy(selT[:], selT_ps[:])

                for g in range(8):
                    half = selT[:, 0:32] if g < 4 else selT[:, 32:64]
                    nc.sync.dma_start(kidx[16 * g:16 * (g + 1), :], half)
                    nc.sync.dma_start(vidx0[16 * g:16 * (g + 1), :], selT[:, 0:32])
                    nc.sync.dma_start(vidx1[16 * g:16 * (g + 1), :], selT[:, 32:64])

            # ---- main loop: sparse attention + chunked projection partials ----
            CHQ = 4                    # query blocks per projection chunk
            CHT = CHQ * BLK            # 512 tokens per chunk
            with tc.tile_pool(name="gather", bufs=3) as gp, \
                 tc.tile_pool(name="escore", bufs=6) as ep, \
                 tc.tile_pool(name="sps", bufs=3, space="PSUM") as spp, \
                 tc.tile_pool(name="ops", bufs=2, space="PSUM") as opp, \
                 tc.tile_pool(name="otp", bufs=2) as otp, \
                 tc.tile_pool(name="prout", bufs=4) as pr, \
                 tc.tile_pool(name="osb", bufs=3) as ob:

                def _emit_proj(c):
                    ot = otp.tile([128, CHT], BF16, tag="ot", name=f"ot_{c}")
                    nc.sync.dma_start_transpose(
                        ot[:], obounce.ap()[c * CHT:(c + 1) * CHT, :])
                    for m in range(KC):
                        pj = spp.tile([128, 1024], F32, tag="s", name=f"pj_{c}_{m}")
                        nc.tensor.matmul(pj[:, 0:CHT],
                                         lhsT=projW_bf[:, m * 128:(m + 1) * 128],
                                         rhs=ot[:], start=True, stop=True)
                        po = pr.tile([128, CHT], F32, tag="po", name=f"po_{c}_{m}")
                        nc.vector.tensor_scalar(po[:], pj[:, 0:CHT],
                                                projb_sb[:, m:m + 1], None,
                                                op0=mybir.AluOpType.add)
                        nc.sync.dma_start(
                            out_ext.ap()[m * 128:(m + 1) * 128, c * CHT:(c + 1) * CHT],
                            po[:])

                for qb in range(NB):
                    kg = gp.tile([128, TOPK, BLK], BF16, tag="kg")
                    nc.gpsimd.ap_gather(kg[:], kT[:], kidx[:, qb:qb + 1],
                                        channels=128, num_elems=NB, d=BLK, num_idxs=TOPK)
                    vg0 = gp.tile([128, TOPK, 66], BF16, tag="vg0")
                    nc.gpsimd.ap_gather(vg0[:], v0[:], vidx0[:, qb:qb + 1],
                                        channels=128, num_elems=NB, d=66, num_idxs=TOPK)
                    vg1 = gp.tile([128, TOPK, 66], BF16, tag="vg1")
                    nc.gpsimd.ap_gather(vg1[:], v1[:], vidx1[:, qb:qb + 1],
                                        channels=128, num_elems=NB, d=66, num_idxs=TOPK)

                    qcol = slice(qb * BLK, (qb + 1) * BLK)
                    etiles = [[None, None], [None, None]]
                    for half in range(2):
                        s0 = spp.tile([128, 1024], F32, tag="s", name=f"s0_{qb}_{half}")
                        s1 = spp.tile([128, 1024], F32, tag="s", name=f"s1_{qb}_{half}")
                        for jj in range(8):
                            j = half * 8 + jj
                            nc.tensor.matmul(s0[:, jj * 128:(jj + 1) * 128],
                                             lhsT=kg[0:64, j, :], rhs=qT[0:64, qcol],
                                             start=True, stop=True)
                            nc.tensor.matmul(s1[:, jj * 128:(jj + 1) * 128],
                                             lhsT=kg[64:128, j, :], rhs=qT[64:128, qcol],
                                             start=True, stop=True)
                        e0 = ep.tile([128, 1024], BF16, tag="e", name=f"e0_{qb}_{half}")
                        e1 = ep.tile([128, 1024], BF16, tag="e", name=f"e1_{qb}_{half}")
                        nc.scalar.activation(e0[:], s0[:],
                                             mybir.ActivationFunctionType.Exp, scale=SCALE)
                        nc.scalar.activation(e1[:], s1[:],
                                             mybir.ActivationFunctionType.Exp, scale=SCALE)
                        etiles[0][half] = e0
                        etiles[1][half] = e1

                    onorm = ob.tile([128, 2 * D], BF16, tag="onorm")
                    for h in range(HPC):
                        vg = vg0 if h == 0 else vg1
                        o_ps = opp.tile([128, D + 1], F32, tag="o")
                        for j in range(TOPK):
                            nc.tensor.matmul(o_ps[:],
                                             lhsT=etiles[h][j // 8][:, (j % 8) * 128:(j % 8 + 1) * 128],
                                             rhs=vg[:, j, 0:D + 1],
                                             start=(j == 0), stop=(j == TOPK - 1))
                        rec = ob.tile([128, 1], F32, tag="rec")
                        nc.vector.reciprocal(rec[:], o_ps[:, D:D + 1])
                        nc.vector.tensor_scalar(onorm[:, h * D:(h + 1) * D],
                                                o_ps[:, 0:D], rec[:], None,
                                                op0=mybir.AluOpType.mult)
                    nc.sync.dma_start(obounce.ap()[qb * BLK:(qb + 1) * BLK, :],
                                      onorm[:])
                    if qb % CHQ == CHQ - 1:
                        _emit_proj(qb // CHQ)

    nc.compile()
    return nc


def _prep_inputs(x, qkv_w, proj_w, proj_b):
    x = np.asarray(x, dtype=np.float32)
    qkv_w = np.asarray(qkv_w, dtype=np.float32)
    proj_w = np.asarray(proj_w, dtype=np.float32)
    proj_b = np.asarray(proj_b, dtype=np.float32)

    xT = np.ascontiguousarray(x[0].T)                      # [C, N]
    ident64 = np.eye(64, dtype=np.float32)
    zero_b = np.zeros((128, 8), dtype=np.float32)
    in_maps = []
    for i in range(NCORES):
        h0 = HPC * i
        rows = []
        for part in range(3):                              # q, k, v row groups
            base = part * C + h0 * D
            rows.append(qkv_w[base:base + HPC * D, :])
        wqkv = np.concatenate(rows, axis=0)                # [384, C]
        cslice = slice(i * 2 * D, (i + 1) * 2 * D)
        in_maps.append({
            "xT": xT,
            "wqkvT": np.ascontiguousarray(wqkv.T),         # [C, 384]
            # [c_local, j]: rows = this core's 128 c-dims, cols = all 1024 j
            "projWT": np.ascontiguousarray(proj_w[:, cslice].T),
            # bias only on core 0 (partials are summed on the host)
            "projb": (np.ascontiguousarray(proj_b.reshape(8, 128).T)
                      if i == 0 else zero_b),
            "ident64": ident64,
        })
    return in_maps


def kernel(x, qkv_w, proj_w, proj_b, _trace=False):
    if "nc" not in _CACHE:
        _CACHE["nc"] = _build()
    nc = _CACHE["nc"]
    in_maps = _prep_inputs(x, qkv_w, proj_w, proj_b)
    res = run_bass_kernel_spmd(nc, in_maps, core_ids=list(range(NCORES)),
                               trace=_trace)
    outT = res.results[0]["out"].astype(np.float32)
    for i in range(1, NCORES):
        outT += res.results[i]["out"]
    out = np.ascontiguousarray(outT.T).reshape(1, N, C).astype(np.float32)
    if _trace:
        _CACHE["last_exec_time_ns"] = res.exec_time_ns
        _CACHE["last_results"] = res
    return out


# revision 15
# speedup vs baseline: 1.5259x; 1.1637x over previous
"""Block-sparse attention (SageAttention-style mean-similarity top-k) on 8 TRN2 NeuronCores.

Sharding: 16 heads tensor-parallel across 8 cores (2 heads/core).
  - qkv weight column-sharded per core (its 2 heads' q/k/v rows, pre-transposed on host)
  - block selection + block-sparse attention fully local per head
  - proj weight row-sharded: each core computes the full-shape PARTIAL product
    o_local @ projW[:, c_slice].T (+ bias on core 0 only); the host unshard step
    sums the 8 partials (the row-parallel reduction).

Per-core device pipeline (bf16 compute, f32 selection):
  x^T f32 -> block sums (DVE) -> qm/km/sim (f32 PE) -> top-16 via max8/max_index
  qkv matmuls (bf16 PE), k kept d-major, v token-major with a ones column
  per query block: ap_gather (GPSIMD ucode, SBUF->SBUF) pulls the 16 selected
  k/v blocks; scores s^T = k_sel^T q (two heads packed in the 128x128 PE array
  via row groups); exp on ACT straight from PSUM; o = (e^T)^T v_sel with the
  gathered ones column yielding the softmax denominator; per-partition
  normalize; chunk-wise DMA-transpose + projection partials streamed out.
"""

import os
import sys

for _p in ("/opt/trn_rl_repo", "/root/.axon_site/_ro/trn_rl_repo"):
    if os.path.isdir(_p) and _p not in sys.path:
        sys.path.insert(0, _p)

import numpy as np

import concourse.bass as bass
import concourse.bacc as bacc
import concourse.tile as tile
import concourse.mybir as mybir
from concourse.bass_utils import run_bass_kernel_spmd
from concourse.library_config import ap_gather as ap_gather_lib

# problem constants
N = 4096          # sequence length
C = 1024          # model dim
H = 16            # heads
D = 64            # head dim
BLK = 128         # block size
NB = N // BLK     # 32 blocks
TOPK = 16         # int(0.5 * NB)
NCORES = 8
HPC = H // NCORES  # 2 heads per core
SCALE = D ** -0.5  # 0.125

F32 = mybir.dt.float32
BF16 = mybir.dt.bfloat16
I16 = mybir.dt.int16
U32 = mybir.dt.uint32

_CACHE = {}


def _build():
    nc = bacc.Bacc("TRN2", target_bir_lowering=False, debug=False,
                   num_devices=NCORES)

    KC = C // 128  # 8 contraction tiles

    xT = nc.dram_tensor("xT", [C, N], F32, kind="ExternalInput")
    wqkvT = nc.dram_tensor("wqkvT", [C, 3 * 2 * D], F32, kind="ExternalInput")
    projWT = nc.dram_tensor("projWT", [2 * D, C], F32, kind="ExternalInput")
    projb = nc.dram_tensor("projb", [128, KC], F32, kind="ExternalInput")
    ident64 = nc.dram_tensor("ident64", [64, 64], F32, kind="ExternalInput")
    out_ext = nc.dram_tensor("out", [C, N], F32, kind="ExternalOutput")

    obounce = nc.dram_tensor("obounce", [N, 2 * D], BF16)

    with tile.TileContext(nc) as tc:
        nc.gpsimd.load_library(ap_gather_lib)

        with tc.tile_pool(name="persist", bufs=1) as pp:
            # ---- weights ----
            wqkv_bf = pp.tile([128, KC, 384], BF16)
            nc.gpsimd.dma_start(
                wqkv_bf[:], wqkvT.ap().rearrange("(a p) m -> p a m", p=128))
            wqk_f32 = pp.tile([128, KC, 256], F32)
            nc.sync.dma_start(
                wqk_f32[:], wqkvT.ap().rearrange("(a p) m -> p a m", p=128)[:, :, 0:256])
            projW_bf = pp.tile([128, C], BF16)          # [c_local, j]
            nc.gpsimd.dma_start(projW_bf[:], projWT.ap())
            projb_sb = pp.tile([128, KC], F32)          # bias for j-tile m in col m
            nc.sync.dma_start(projb_sb[:], projb.ap())
            id64 = pp.tile([64, 64], F32)
            nc.sync.dma_start(id64[:], ident64.ap())

            # ---- x: one f32 read; DVE block sums, ACT cast -> xbf ----
            xbf = pp.tile([128, KC, N], BF16)
            xm = pp.tile([128, KC, NB], F32)
            with tc.tile_pool(name="xload", bufs=2) as xp:
                for kc in range(KC):
                    xf = xp.tile([128, N], F32, tag="xf")
                    nc.sync.dma_start(xf[:], xT.ap()[kc * 128:(kc + 1) * 128, :])
                    nc.vector.tensor_reduce(
                        xm[:, kc, :], xf[:].rearrange("p (b t) -> p b t", t=BLK),
                        axis=mybir.AxisListType.X, op=mybir.AluOpType.add)
                    nc.scalar.copy(xbf[:, kc, :], xf[:])

            # ---- block-mean similarity + top-k selection (f32) ----
            kidx = pp.tile([128, NB], I16)
            vidx0 = pp.tile([128, NB], I16)
            vidx1 = pp.tile([128, NB], I16)
            with tc.tile_pool(name="selps", bufs=2, space="PSUM") as sp, \
                 tc.tile_pool(name="selsb", bufs=2) as sb:
                qm_ps = sp.tile([128, NB], F32, tag="qkm")
                km_ps = sp.tile([128, NB], F32, tag="qkm")
                for kc in range(KC):
                    nc.tensor.matmul(qm_ps[:], lhsT=wqk_f32[:, kc, 0:128],
                                     rhs=xm[:, kc, :], start=(kc == 0), stop=(kc == KC - 1))
                for kc in range(KC):
                    nc.tensor.matmul(km_ps[:], lhsT=wqk_f32[:, kc, 128:256],
                                     rhs=xm[:, kc, :], start=(kc == 0), stop=(kc == KC - 1))
                qm_sb = sb.tile([128, NB], F32, tag="qm")
                km_sb = sb.tile([128, NB], F32, tag="km")
                nc.scalar.copy(qm_sb[:], qm_ps[:])
                nc.scalar.copy(km_sb[:], km_ps[:])

                sim_ps = sp.tile([64, NB], F32, tag="sim")
                for h in range(HPC):
                    nc.tensor.matmul(sim_ps[h * 32:(h + 1) * 32, :],
                                     lhsT=qm_sb[h * 64:(h + 1) * 64, :],
                                     rhs=km_sb[h * 64:(h + 1) * 64, :],
                                     start=True, stop=True)
                sim2 = sb.tile([64, NB], F32, tag="sim2")
                nc.vector.tensor_copy(sim2[:], sim_ps[:])

                vals0 = sb.tile([64, 8], F32, tag="v0")
                idx0 = sb.tile([64, 8], U32, tag="i0")
                pun = sb.tile([64, NB], F32, tag="pun")
                vals1 = sb.tile([64, 8], F32, tag="v1")
                idx1 = sb.tile([64, 8], U32, tag="i1")
                nc.vector.max(vals0[:], sim2[:])
                nc.vector.max_index(idx0[:], vals0[:], sim2[:])
                nc.vector.match_replace(out=pun[:], in_to_replace=vals0[:],
                                        in_values=sim2[:], imm_value=-1e30)
                nc.vector.max(vals1[:], pun[:])
                nc.vector.max_index(idx1[:], vals1[:], pun[:])

                idxf = sb.tile([64, TOPK], F32, tag="idxf")
                nc.vector.tensor_copy(idxf[:, 0:8], idx0[:])
                nc.vector.tensor_copy(idxf[:, 8:16], idx1[:])

                selT_ps = sp.tile([TOPK, 64], F32, tag="selT")
                nc.tensor.transpose(selT_ps[:], idxf[:], id64[:])
                selT = sb.tile([TOPK, 64], I16, tag="selTsb")
                nc.vector.tensor_copy(selT[:], selT_ps[:])

                for g in range(8):
                    half = selT[:, 0:32] if g < 4 else selT[:, 32:64]
                    nc.sync.dma_start(kidx[16 * g:16 * (g + 1), :], half)
                    nc.sync.dma_start(vidx0[16 * g:16 * (g + 1), :], selT[:, 0:32])
                    nc.sync.dma_start(vidx1[16 * g:16 * (g + 1), :], selT[:, 32:64])

            # ---- QKV (bf16) ----
            qT = pp.tile([128, N], BF16)
            kT = pp.tile([128, NB, BLK], BF16)   # contiguous == [128, N]
            v0 = pp.tile([128, NB, 66], BF16)
            v1 = pp.tile([128, NB, 66], BF16)
            nc.vector.memset(v0[:, :, 64:66], 0.0)
            nc.vector.memset(v1[:, :, 64:66], 0.0)
            nc.vector.memset(v0[:, :, 64:65], 1.0)
            nc.vector.memset(v1[:, :, 64:65], 1.0)

            with tc.tile_pool(name="qkps", bufs=3, space="PSUM") as qp:
                for mt in (0, 1):
                    for nch in range(8):
                        ps = qp.tile([128, 512], F32, tag="qk")
                        for kc in range(KC):
                            nc.tensor.matmul(
                                ps[:], lhsT=wqkv_bf[:, kc, mt * 128:(mt + 1) * 128],
                                rhs=xbf[:, kc, nch * 512:(nch + 1) * 512],
                                start=(kc == 0), stop=(kc == KC - 1))
                        if mt == 0:
                            nc.scalar.copy(qT[:, nch * 512:(nch + 1) * 512], ps[:])
                        else:
                            nc.scalar.copy(
                                kT[:].rearrange("p a b -> p (a b)")[:, nch * 512:(nch + 1) * 512],
                                ps[:])

            with tc.tile_pool(name="vps", bufs=3, space="PSUM") as vp:
                for nt in range(NB):
                    ps = vp.tile([128, 128], F32, tag="v")
                    for kc in range(KC):
                        nc.tensor.matmul(ps[:], lhsT=xbf[:, kc, nt * 128:(nt + 1) * 128],
                                         rhs=wqkv_bf[:, kc, 256:384],
                                         start=(kc == 0), stop=(kc == KC - 1))
                    nc.vector.tensor_copy(v0[:, nt, 0:64], ps[:, 0:64])
                    nc.vector.tensor_copy(v1[:, nt, 0:64], ps[:, 64:128])

            # ---- main loop: sparse attention + chunked projection partials ----
            CHQ = 4                    # query blocks per projection chunk
            CHT = CHQ * BLK            # 512 tokens per chunk
            with tc.tile_pool(name="gather", bufs=3) as gp, \
                 tc.tile_pool(name="escore", bufs=6) as ep, \
                 tc.tile_pool(name="sps", bufs=3, space="PSUM") as spp, \
                 tc.tile_pool(name="ops", bufs=2, space="PSUM") as opp, \
                 tc.tile_pool(name="otp", bufs=2) as otp, \
                 tc.tile_pool(name="prout", bufs=4) as pr, \
                 tc.tile_pool(name="osb", bufs=3) as ob:

                def _emit_proj(c):
                    ot = otp.tile([128, CHT], BF16, tag="ot", name=f"ot_{c}")
                    nc.sync.dma_start_transpose(
                        ot[:], obounce.ap()[c * CHT:(c + 1) * CHT, :])
                    for m in range(KC):
                        pj = spp.tile([128, 1024], F32, tag="s", name=f"pj_{c}_{m}")
                        nc.tensor.matmul(pj[:, 0:CHT],
                                         lhsT=projW_bf[:, m * 128:(m + 1) * 128],
                                         rhs=ot[:], start=True, stop=True)
                        po = pr.tile([128, CHT], F32, tag="po", name=f"po_{c}_{m}")
                        nc.vector.tensor_scalar(po[:], pj[:, 0:CHT],
                                                projb_sb[:, m:m + 1], None,
                                                op0=mybir.AluOpType.add)
                        nc.sync.dma_start(
                            out_ext.ap()[m * 128:(m + 1) * 128, c * CHT:(c + 1) * CHT],
                            po[:])

                for qb in range(NB):
                    kg = gp.tile([128, TOPK, BLK], BF16, tag="kg")
                    nc.gpsimd.ap_gather(kg[:], kT[:], kidx[:, qb:qb + 1],
                                        channels=128, num_elems=NB, d=BLK, num_idxs=TOPK)
                    vg0 = gp.tile([128, TOPK, 66], BF16, tag="vg0")
                    nc.gpsimd.ap_gather(vg0[:], v0[:], vidx0[:, qb:qb + 1],
                                        channels=128, num_elems=NB, d=66, num_idxs=TOPK)
                    vg1 = gp.tile([128, TOPK, 66], BF16, tag="vg1")
                    nc.gpsimd.ap_gather(vg1[:], v1[:], vidx1[:, qb:qb + 1],
                                        channels=128, num_elems=NB, d=66, num_idxs=TOPK)

                    qcol = slice(qb * BLK, (qb + 1) * BLK)
                    etiles = [[None, None], [None, None]]
                    for half in range(2):
                        s0 = spp.tile([128, 1024], F32, tag="s", name=f"s0_{qb}_{half}")
                        s1 = spp.tile([128, 1024], F32, tag="s", name=f"s1_{qb}_{half}")
                        for jj in range(8):
                            j = half * 8 + jj
                            nc.tensor.matmul(s0[:, jj * 128:(jj + 1) * 128],
                                             lhsT=kg[0:64, j, :], rhs=qT[0:64, qcol],
                                             start=True, stop=True)
                            nc.tensor.matmul(s1[:, jj * 128:(jj + 1) * 128],
                                             lhsT=kg[64:128, j, :], rhs=qT[64:128, qcol],
                                             start=True, stop=True)
                        e0 = ep.tile([128, 1024], BF16, tag="e", name=f"e0_{qb}_{half}")
                        e1 = ep.tile([128, 1024], BF16, tag="e", name=f"e1_{qb}_{half}")
                        nc.scalar.activation(e0[:], s0[:],
                                             mybir.ActivationFunctionType.Exp, scale=SCALE)
                        nc.scalar.activation(e1[:], s1[:],
                                             mybir.ActivationFunctionType.Exp, scale=SCALE)
                        etiles[0][half] = e0
                        etiles[1][half] = e1

                    onorm = ob.tile([128, 2 * D], BF16, tag="onorm")
                    for h in range(HPC):
                        vg = vg0 if h == 0 else vg1
                        o_ps = opp.tile([128, D + 1], F32, tag="o")
                        for j in range(TOPK):
                            nc.tensor.matmul(o_ps[:],
                                             lhsT=etiles[h][j // 8][:, (j % 8) * 128:(j % 8 + 1) * 128],
                                             rhs=vg[:, j, 0:D + 1],
                                             start=(j == 0), stop=(j == TOPK - 1))
                        rec = ob.tile([128, 1], F32, tag="rec")
                        nc.vector.reciprocal(rec[:], o_ps[:, D:D + 1])
                        nc.vector.tensor_scalar(onorm[:, h * D:(h + 1) * D],
                                                o_ps[:, 0:D], rec[:], None,
                                                op0=mybir.AluOpType.mult)
                    nc.sync.dma_start(obounce.ap()[qb * BLK:(qb + 1) * BLK, :],
                                      onorm[:])
                    if qb % CHQ == CHQ - 1:
                        _emit_proj(qb // CHQ)

    nc.compile()
    return nc


def _prep_inputs(x, qkv_w, proj_w, proj_b):
    x = np.asarray(x, dtype=np.float32)
    qkv_w = np.asarray(qkv_w, dtype=np.float32)
    proj_w = np.asarray(proj_w, dtype=np.float32)
    proj_b = np.asarray(proj_b, dtype=np.float32)

    xT = np.ascontiguousarray(x[0].T)                      # [C, N]
    ident64 = np.eye(64, dtype=np.float32)
    zero_b = np.zeros((128, 8), dtype=np.float32)
    in_maps = []
    for i in range(NCORES):
        h0 = HPC * i
        rows = []
        for part in range(3):                              # q, k, v row groups
            base = part * C + h0 * D
            rows.append(qkv_w[base:base + HPC * D, :])
        wqkv = np.concatenate(rows, axis=0)                # [384, C]
        cslice = slice(i * 2 * D, (i + 1) * 2 * D)
        in_maps.append({
            "xT": xT,
            "wqkvT": np.ascontiguousarray(wqkv.T),         # [C, 384]
            # [c_local, j]: rows = this core's 128 c-dims, cols = all 1024 j
            "projWT": np.ascontiguousarray(proj_w[:, cslice].T),
            # bias only on core 0 (partials are summed on the host)
            "projb": (np.ascontiguousarray(proj_b.reshape(8, 128).T)
                      if i == 0 else zero_b),
            "ident64": ident64,
        })
    return in_maps


def kernel(x, qkv_w, proj_w, proj_b, _trace=False):
    if "nc" not in _CACHE:
        _CACHE["nc"] = _build()
    nc = _CACHE["nc"]
    in_maps = _prep_inputs(x, qkv_w, proj_w, proj_b)
    res = run_bass_kernel_spmd(nc, in_maps, core_ids=list(range(NCORES)),
                               trace=_trace)
    outT = res.results[0]["out"].astype(np.float32)
    for i in range(1, NCORES):
        outT += res.results[i]["out"]
    out = np.ascontiguousarray(outT.T).reshape(1, N, C).astype(np.float32)
    if _trace:
        _CACHE["last_exec_time_ns"] = res.exec_time_ns
        _CACHE["last_results"] = res
    return out


# revision 16
# speedup vs baseline: 1.5442x; 1.0120x over previous
"""Block-sparse attention (SageAttention-style mean-similarity top-k) on 8 TRN2 NeuronCores.

Sharding: 16 heads tensor-parallel across 8 cores (2 heads/core).
  - qkv weight column-sharded per core (its 2 heads' q/k/v rows, pre-transposed on host)
  - block selection + block-sparse attention fully local per head
  - proj weight row-sharded: each core computes the full-shape PARTIAL product
    o_local @ projW[:, c_slice].T (+ bias on core 0 only); the host unshard step
    sums the 8 partials (the row-parallel reduction).

Per-core device pipeline (bf16 compute, f32 selection):
  x^T f32 -> block sums (DVE) -> qm/km/sim (f32 PE) -> top-16 via max8/max_index
  qkv matmuls (bf16 PE), k kept d-major, v token-major with a ones column
  per query block: ap_gather (GPSIMD ucode, SBUF->SBUF) pulls the 16 selected
  k/v blocks; scores s^T = k_sel^T q (two heads packed in the 128x128 PE array
  via row groups); exp on ACT straight from PSUM; o = (e^T)^T v_sel with the
  gathered ones column yielding the softmax denominator; per-partition
  normalize; chunk-wise DMA-transpose + projection partials streamed out.
"""

import os
import sys

for _p in ("/opt/trn_rl_repo", "/root/.axon_site/_ro/trn_rl_repo"):
    if os.path.isdir(_p) and _p not in sys.path:
        sys.path.insert(0, _p)

import numpy as np

import concourse.bass as bass
import concourse.bacc as bacc
import concourse.tile as tile
import concourse.mybir as mybir
from concourse.bass_utils import run_bass_kernel_spmd
from concourse.library_config import ap_gather as ap_gather_lib

# problem constants
N = 4096          # sequence length
C = 1024          # model dim
H = 16            # heads
D = 64            # head dim
BLK = 128         # block size
NB = N // BLK     # 32 blocks
TOPK = 16         # int(0.5 * NB)
NCORES = 8
HPC = H // NCORES  # 2 heads per core
SCALE = D ** -0.5  # 0.125

F32 = mybir.dt.float32
BF16 = mybir.dt.bfloat16
I16 = mybir.dt.int16
U32 = mybir.dt.uint32

_CACHE = {}


def _build():
    nc = bacc.Bacc("TRN2", target_bir_lowering=False, debug=False,
                   num_devices=NCORES)

    KC = C // 128  # 8 contraction tiles

    xT = nc.dram_tensor("xT", [C, N], F32, kind="ExternalInput")
    wqkvT = nc.dram_tensor("wqkvT", [C, 3 * 2 * D], F32, kind="ExternalInput")
    projWT = nc.dram_tensor("projWT", [2 * D, C], F32, kind="ExternalInput")
    projb = nc.dram_tensor("projb", [128, KC], F32, kind="ExternalInput")
    ident64 = nc.dram_tensor("ident64", [64, 64], F32, kind="ExternalInput")
    out_ext = nc.dram_tensor("out", [C, N], F32, kind="ExternalOutput")

    obounce = nc.dram_tensor("obounce", [N, 2 * D], BF16)

    with tile.TileContext(nc) as tc:
        nc.gpsimd.load_library(ap_gather_lib)

        with tc.tile_pool(name="persist", bufs=1) as pp:
            # ---- weights ----
            wqkv_bf = pp.tile([128, KC, 384], BF16)
            nc.gpsimd.dma_start(
                wqkv_bf[:], wqkvT.ap().rearrange("(a p) m -> p a m", p=128))
            wqk_f32 = pp.tile([128, KC, 256], F32)
            nc.sync.dma_start(
                wqk_f32[:], wqkvT.ap().rearrange("(a p) m -> p a m", p=128)[:, :, 0:256])
            projW_bf = pp.tile([128, C], BF16)          # [c_local, j]
            nc.gpsimd.dma_start(projW_bf[:], projWT.ap())
            projb_sb = pp.tile([128, KC], F32)          # bias for j-tile m in col m
            nc.sync.dma_start(projb_sb[:], projb.ap())
            id64 = pp.tile([64, 64], F32)
            nc.sync.dma_start(id64[:], ident64.ap())

            # ---- x: one f32 read; DVE block sums, ACT cast -> xbf ----
            xbf = pp.tile([128, KC, N], BF16)
            xm = pp.tile([128, KC, NB], F32)
            with tc.tile_pool(name="xload", bufs=2) as xp:
                for kc in range(KC):
                    xf = xp.tile([128, N], F32, tag="xf")
                    nc.sync.dma_start(xf[:], xT.ap()[kc * 128:(kc + 1) * 128, :])
                    nc.vector.tensor_reduce(
                        xm[:, kc, :], xf[:].rearrange("p (b t) -> p b t", t=BLK),
                        axis=mybir.AxisListType.X, op=mybir.AluOpType.add)
                    nc.scalar.copy(xbf[:, kc, :], xf[:])

            # ---- block-mean similarity + top-k selection (f32) ----
            kidx = pp.tile([128, NB], I16)
            vidx0 = pp.tile([128, NB], I16)
            vidx1 = pp.tile([128, NB], I16)
            with tc.tile_pool(name="selps", bufs=2, space="PSUM") as sp, \
                 tc.tile_pool(name="selsb", bufs=2) as sb:
                qm_ps = sp.tile([128, NB], F32, tag="qkm")
                km_ps = sp.tile([128, NB], F32, tag="qkm")
                for kc in range(KC):
                    nc.tensor.matmul(qm_ps[:], lhsT=wqk_f32[:, kc, 0:128],
                                     rhs=xm[:, kc, :], start=(kc == 0), stop=(kc == KC - 1))
                for kc in range(KC):
                    nc.tensor.matmul(km_ps[:], lhsT=wqk_f32[:, kc, 128:256],
                                     rhs=xm[:, kc, :], start=(kc == 0), stop=(kc == KC - 1))
                qm_sb = sb.tile([128, NB], F32, tag="qm")
                km_sb = sb.tile([128, NB], F32, tag="km")
                nc.scalar.copy(qm_sb[:], qm_ps[:])
                nc.scalar.copy(km_sb[:], km_ps[:])

                sim_ps = sp.tile([64, NB], F32, tag="sim")
                for h in range(HPC):
                    nc.tensor.matmul(sim_ps[h * 32:(h + 1) * 32, :],
                                     lhsT=qm_sb[h * 64:(h + 1) * 64, :],
                                     rhs=km_sb[h * 64:(h + 1) * 64, :],
                                     start=True, stop=True)
                sim2 = sb.tile([64, NB], F32, tag="sim2")
                nc.vector.tensor_copy(sim2[:], sim_ps[:])

                vals0 = sb.tile([64, 8], F32, tag="v0")
                idx0 = sb.tile([64, 8], U32, tag="i0")
                pun = sb.tile([64, NB], F32, tag="pun")
                vals1 = sb.tile([64, 8], F32, tag="v1")
                idx1 = sb.tile([64, 8], U32, tag="i1")
                nc.vector.max(vals0[:], sim2[:])
                nc.vector.max_index(idx0[:], vals0[:], sim2[:])
                nc.vector.match_replace(out=pun[:], in_to_replace=vals0[:],
                                        in_values=sim2[:], imm_value=-1e30)
                nc.vector.max(vals1[:], pun[:])
                nc.vector.max_index(idx1[:], vals1[:], pun[:])

                idxf = sb.tile([64, TOPK], F32, tag="idxf")
                nc.vector.tensor_copy(idxf[:, 0:8], idx0[:])
                nc.vector.tensor_copy(idxf[:, 8:16], idx1[:])

                selT_ps = sp.tile([TOPK, 64], F32, tag="selT")
                nc.tensor.transpose(selT_ps[:], idxf[:], id64[:])
                selT = sb.tile([TOPK, 64], I16, tag="selTsb")
                nc.vector.tensor_copy(selT[:], selT_ps[:])

                for g in range(8):
                    half = selT[:, 0:32] if g < 4 else selT[:, 32:64]
                    nc.sync.dma_start(kidx[16 * g:16 * (g + 1), :], half)
                    nc.sync.dma_start(vidx0[16 * g:16 * (g + 1), :], selT[:, 0:32])
                    nc.sync.dma_start(vidx1[16 * g:16 * (g + 1), :], selT[:, 32:64])

            # ---- QKV (bf16) ----
            qT = pp.tile([128, N], BF16)
            kT = pp.tile([128, NB, BLK], BF16)   # contiguous == [128, N]
            v0 = pp.tile([128, NB, 66], BF16)
            v1 = pp.tile([128, NB, 66], BF16)
            nc.vector.memset(v0[:, :, 64:66], 0.0)
            nc.vector.memset(v1[:, :, 64:66], 0.0)
            nc.vector.memset(v0[:, :, 64:65], 1.0)
            nc.vector.memset(v1[:, :, 64:65], 1.0)

            with tc.tile_pool(name="qkps", bufs=3, space="PSUM") as qp:
                for mt in (0, 1):
                    for nch in range(8):
                        ps = qp.tile([128, 512], F32, tag="qk")
                        for kc in range(KC):
                            nc.tensor.matmul(
                                ps[:], lhsT=wqkv_bf[:, kc, mt * 128:(mt + 1) * 128],
                                rhs=xbf[:, kc, nch * 512:(nch + 1) * 512],
                                start=(kc == 0), stop=(kc == KC - 1))
                        if mt == 0:
                            nc.scalar.copy(qT[:, nch * 512:(nch + 1) * 512], ps[:])
                        else:
                            nc.scalar.copy(
                                kT[:].rearrange("p a b -> p (a b)")[:, nch * 512:(nch + 1) * 512],
                                ps[:])

            with tc.tile_pool(name="vps", bufs=3, space="PSUM") as vp:
                for nt in range(NB):
                    ps = vp.tile([128, 128], F32, tag="v")
                    for kc in range(KC):
                        nc.tensor.matmul(ps[:], lhsT=xbf[:, kc, nt * 128:(nt + 1) * 128],
                                         rhs=wqkv_bf[:, kc, 256:384],
                                         start=(kc == 0), stop=(kc == KC - 1))
                    nc.vector.tensor_copy(v0[:, nt, 0:64], ps[:, 0:64])
                    nc.vector.tensor_copy(v1[:, nt, 0:64], ps[:, 64:128])

            # ---- main loop: sparse attention + chunked projection partials ----
            CHQ = 4                    # query blocks per projection chunk
            CHT = CHQ * BLK            # 512 tokens per chunk
            with tc.tile_pool(name="gather", bufs=4) as gp, \
                 tc.tile_pool(name="escore", bufs=8) as ep, \
                 tc.tile_pool(name="sps", bufs=3, space="PSUM") as spp, \
                 tc.tile_pool(name="ops", bufs=2, space="PSUM") as opp, \
                 tc.tile_pool(name="otp", bufs=2) as otp, \
                 tc.tile_pool(name="prout", bufs=4) as pr, \
                 tc.tile_pool(name="osb", bufs=3) as ob:

                def _emit_proj(c):
                    ot = otp.tile([128, CHT], BF16, tag="ot", name=f"ot_{c}")
                    nc.sync.dma_start_transpose(
                        ot[:], obounce.ap()[c * CHT:(c + 1) * CHT, :])
                    for m in range(KC):
                        pj = spp.tile([128, 1024], F32, tag="s", name=f"pj_{c}_{m}")
                        nc.tensor.matmul(pj[:, 0:CHT],
                                         lhsT=projW_bf[:, m * 128:(m + 1) * 128],
                                         rhs=ot[:], start=True, stop=True)
                        po = pr.tile([128, CHT], F32, tag="po", name=f"po_{c}_{m}")
                        nc.vector.tensor_scalar(po[:], pj[:, 0:CHT],
                                                projb_sb[:, m:m + 1], None,
                                                op0=mybir.AluOpType.add)
                        nc.sync.dma_start(
                            out_ext.ap()[m * 128:(m + 1) * 128, c * CHT:(c + 1) * CHT],
                            po[:])

                for qb in range(NB):
                    kg = gp.tile([128, TOPK, BLK], BF16, tag="kg")
                    nc.gpsimd.ap_gather(kg[:], kT[:], kidx[:, qb:qb + 1],
                                        channels=128, num_elems=NB, d=BLK, num_idxs=TOPK)
                    vg0 = gp.tile([128, TOPK, 66], BF16, tag="vg0")
                    nc.gpsimd.ap_gather(vg0[:], v0[:], vidx0[:, qb:qb + 1],
                                        channels=128, num_elems=NB, d=66, num_idxs=TOPK)
                    vg1 = gp.tile([128, TOPK, 66], BF16, tag="vg1")
                    nc.gpsimd.ap_gather(vg1[:], v1[:], vidx1[:, qb:qb + 1],
                                        channels=128, num_elems=NB, d=66, num_idxs=TOPK)

                    qcol = slice(qb * BLK, (qb + 1) * BLK)
                    etiles = [[None, None], [None, None]]
                    for half in range(2):
                        s0 = spp.tile([128, 1024], F32, tag="s", name=f"s0_{qb}_{half}")
                        s1 = spp.tile([128, 1024], F32, tag="s", name=f"s1_{qb}_{half}")
                        for jj in range(8):
                            j = half * 8 + jj
                            nc.tensor.matmul(s0[:, jj * 128:(jj + 1) * 128],
                                             lhsT=kg[0:64, j, :], rhs=qT[0:64, qcol],
                                             start=True, stop=True)
                            nc.tensor.matmul(s1[:, jj * 128:(jj + 1) * 128],
                                             lhsT=kg[64:128, j, :], rhs=qT[64:128, qcol],
                                             start=True, stop=True)
                        e0 = ep.tile([128, 1024], BF16, tag="e", name=f"e0_{qb}_{half}")
                        e1 = ep.tile([128, 1024], BF16, tag="e", name=f"e1_{qb}_{half}")
                        nc.scalar.activation(e0[:], s0[:],
                                             mybir.ActivationFunctionType.Exp, scale=SCALE)
                        nc.scalar.activation(e1[:], s1[:],
                                             mybir.ActivationFunctionType.Exp, scale=SCALE)
                        etiles[0][half] = e0
                        etiles[1][half] = e1

                    onorm = ob.tile([128, 2 * D], BF16, tag="onorm")
                    for h in range(HPC):
                        vg = vg0 if h == 0 else vg1
                        o_ps = opp.tile([128, D + 1], F32, tag="o")
                        for j in range(TOPK):
                            nc.tensor.matmul(o_ps[:],
                                             lhsT=etiles[h][j // 8][:, (j % 8) * 128:(j % 8 + 1) * 128],
                                             rhs=vg[:, j, 0:D + 1],
                                             start=(j == 0), stop=(j == TOPK - 1))
                        rec = ob.tile([128, 1], F32, tag="rec")
                        nc.vector.reciprocal(rec[:], o_ps[:, D:D + 1])
                        nc.vector.tensor_scalar(onorm[:, h * D:(h + 1) * D],
                                                o_ps[:, 0:D], rec[:], None,
                                                op0=mybir.AluOpType.mult)
                    nc.sync.dma_start(obounce.ap()[qb * BLK:(qb + 1) * BLK, :],
                                      onorm[:])
                    if qb % CHQ == CHQ - 1:
                        _emit_proj(qb // CHQ)

    nc.compile()
    return nc


def _prep_inputs(x, qkv_w, proj_w, proj_b):
    x = np.asarray(x, dtype=np.float32)
    qkv_w = np.asarray(qkv_w, dtype=np.float32)
    proj_w = np.asarray(proj_w, dtype=np.float32)
    proj_b = np.asarray(proj_b, dtype=np.float32)

    xT = np.ascontiguousarray(x[0].T)                      # [C, N]
    ident64 = np.eye(64, dtype=np.float32)
    zero_b = np.zeros((128, 8), dtype=np.float32)
    in_maps = []
    for i in range(NCORES):
        h0 = HPC * i
        rows = []
        for part in range(3):                              # q, k, v row groups
            base = part * C + h0 * D
            rows.append(qkv_w[base:base + HPC * D, :])
        wqkv = np.concatenate(rows, axis=0)                # [384, C]
        cslice = slice(i * 2 * D, (i + 1) * 2 * D)
        in_maps.append({
            "xT": xT,
            "wqkvT": np.ascontiguousarray(wqkv.T),         # [C, 384]
            # [c_local, j]: rows = this core's 128 c-dims, cols = all 1024 j
            "projWT": np.ascontiguousarray(proj_w[:, cslice].T),
            # bias only on core 0 (partials are summed on the host)
            "projb": (np.ascontiguousarray(proj_b.reshape(8, 128).T)
                      if i == 0 else zero_b),
            "ident64": ident64,
        })
    return in_maps


def kernel(x, qkv_w, proj_w, proj_b, _trace=False):
    if "nc" not in _CACHE:
        _CACHE["nc"] = _build()
    nc = _CACHE["nc"]
    in_maps = _prep_inputs(x, qkv_w, proj_w, proj_b)
    res = run_bass_kernel_spmd(nc, in_maps, core_ids=list(range(NCORES)),
                               trace=_trace)
    outT = res.results[0]["out"].astype(np.float32)
    for i in range(1, NCORES):
        outT += res.results[i]["out"]
    out = np.ascontiguousarray(outT.T).reshape(1, N, C).astype(np.float32)
    if _trace:
        _CACHE["last_exec_time_ns"] = res.exec_time_ns
        _CACHE["last_results"] = res
    return out


# revision 17
# speedup vs baseline: 1.6369x; 1.0600x over previous
"""Block-sparse attention (SageAttention-style mean-similarity top-k) on 8 TRN2 NeuronCores.

Sharding: 16 heads tensor-parallel across 8 cores (2 heads/core).
  - qkv weight column-sharded per core (its 2 heads' q/k/v rows, pre-transposed on host)
  - block selection + block-sparse attention fully local per head
  - proj weight row-sharded: each core computes the full-shape PARTIAL product
    o_local @ projW[:, c_slice].T (+ bias on core 0 only); the host unshard step
    sums the 8 partials (the row-parallel reduction).

Per-core device pipeline (bf16 compute, f32 selection):
  x^T f32 -> block sums (DVE) -> qm/km/sim (f32 PE) -> top-16 via max8/max_index
  qkv matmuls (bf16 PE), k kept d-major, v token-major with a ones column
  per query block: ap_gather (GPSIMD ucode, SBUF->SBUF) pulls the 16 selected
  k/v blocks; scores s^T = k_sel^T q (two heads packed in the 128x128 PE array
  via row groups); exp on ACT straight from PSUM; o = (e^T)^T v_sel with the
  gathered ones column yielding the softmax denominator; per-partition
  normalize; chunk-wise DMA-transpose + projection partials streamed out.
"""

import os
import sys

for _p in ("/opt/trn_rl_repo", "/root/.axon_site/_ro/trn_rl_repo"):
    if os.path.isdir(_p) and _p not in sys.path:
        sys.path.insert(0, _p)

import numpy as np

import concourse.bass as bass
import concourse.bacc as bacc
import concourse.tile as tile
import concourse.mybir as mybir
from concourse.bass_utils import run_bass_kernel_spmd
from concourse.library_config import ap_gather as ap_gather_lib

# problem constants
N = 4096          # sequence length
C = 1024          # model dim
H = 16            # heads
D = 64            # head dim
BLK = 128         # block size
NB = N // BLK     # 32 blocks
TOPK = 16         # int(0.5 * NB)
NCORES = 8
HPC = H // NCORES  # 2 heads per core
SCALE = D ** -0.5  # 0.125

F32 = mybir.dt.float32
BF16 = mybir.dt.bfloat16
I16 = mybir.dt.int16
U32 = mybir.dt.uint32

_CACHE = {}


def _build():
    nc = bacc.Bacc("TRN2", target_bir_lowering=False, debug=False,
                   num_devices=NCORES)

    KC = C // 128  # 8 contraction tiles

    xT = nc.dram_tensor("xT", [C, N], F32, kind="ExternalInput")
    wqkvT = nc.dram_tensor("wqkvT", [C, 3 * 2 * D], F32, kind="ExternalInput")
    projWT = nc.dram_tensor("projWT", [2 * D, C], F32, kind="ExternalInput")
    projb = nc.dram_tensor("projb", [128, KC], F32, kind="ExternalInput")
    ident64 = nc.dram_tensor("ident64", [64, 64], F32, kind="ExternalInput")
    out_ext = nc.dram_tensor("out", [C, N], F32, kind="ExternalOutput")

    obounce = nc.dram_tensor("obounce", [N, 2 * D], BF16)

    with tile.TileContext(nc) as tc:
        nc.gpsimd.load_library(ap_gather_lib)

        with tc.tile_pool(name="persist", bufs=1) as pp:
            # ---- weights ----
            wqkv_bf = pp.tile([128, KC, 384], BF16)
            nc.gpsimd.dma_start(
                wqkv_bf[:], wqkvT.ap().rearrange("(a p) m -> p a m", p=128))
            wqk_f32 = pp.tile([128, KC, 256], F32)
            nc.sync.dma_start(
                wqk_f32[:], wqkvT.ap().rearrange("(a p) m -> p a m", p=128)[:, :, 0:256])
            projW_bf = pp.tile([128, C], BF16)          # [c_local, j]
            nc.gpsimd.dma_start(projW_bf[:], projWT.ap())
            projb_sb = pp.tile([128, KC], F32)          # bias for j-tile m in col m
            nc.sync.dma_start(projb_sb[:], projb.ap())
            id64 = pp.tile([64, 64], F32)
            nc.sync.dma_start(id64[:], ident64.ap())

            # ---- x: one f32 read; DVE block sums, ACT cast -> xbf ----
            xbf = pp.tile([128, KC, N], BF16)
            xm = pp.tile([128, KC, NB], F32)
            with tc.tile_pool(name="xload", bufs=2) as xp:
                for kc in range(KC):
                    xf = xp.tile([128, N], F32, tag="xf")
                    nc.sync.dma_start(xf[:], xT.ap()[kc * 128:(kc + 1) * 128, :])
                    nc.vector.tensor_reduce(
                        xm[:, kc, :], xf[:].rearrange("p (b t) -> p b t", t=BLK),
                        axis=mybir.AxisListType.X, op=mybir.AluOpType.add)
                    nc.scalar.copy(xbf[:, kc, :], xf[:])

            # ---- block-mean similarity + top-k selection (f32) ----
            kidx = pp.tile([128, NB], I16)
            vidx0 = pp.tile([128, NB], I16)
            vidx1 = pp.tile([128, NB], I16)
            with tc.tile_pool(name="selps", bufs=2, space="PSUM") as sp, \
                 tc.tile_pool(name="selsb", bufs=2) as sb:
                qm_ps = sp.tile([128, NB], F32, tag="qkm")
                km_ps = sp.tile([128, NB], F32, tag="qkm")
                for kc in range(KC):
                    nc.tensor.matmul(qm_ps[:], lhsT=wqk_f32[:, kc, 0:128],
                                     rhs=xm[:, kc, :], start=(kc == 0), stop=(kc == KC - 1))
                for kc in range(KC):
                    nc.tensor.matmul(km_ps[:], lhsT=wqk_f32[:, kc, 128:256],
                                     rhs=xm[:, kc, :], start=(kc == 0), stop=(kc == KC - 1))
                qm_sb = sb.tile([128, NB], F32, tag="qm")
                km_sb = sb.tile([128, NB], F32, tag="km")
                nc.scalar.copy(qm_sb[:], qm_ps[:])
                nc.scalar.copy(km_sb[:], km_ps[:])

                sim_ps = sp.tile([64, NB], F32, tag="sim")
                for h in range(HPC):
                    nc.tensor.matmul(sim_ps[h * 32:(h + 1) * 32, :],
                                     lhsT=qm_sb[h * 64:(h + 1) * 64, :],
                                     rhs=km_sb[h * 64:(h + 1) * 64, :],
                                     start=True, stop=True)
                sim2 = sb.tile([64, NB], F32, tag="sim2")
                nc.vector.tensor_copy(sim2[:], sim_ps[:])

                vals0 = sb.tile([64, 8], F32, tag="v0")
                idx0 = sb.tile([64, 8], U32, tag="i0")
                pun = sb.tile([64, NB], F32, tag="pun")
                vals1 = sb.tile([64, 8], F32, tag="v1")
                idx1 = sb.tile([64, 8], U32, tag="i1")
                nc.vector.max(vals0[:], sim2[:])
                nc.vector.max_index(idx0[:], vals0[:], sim2[:])
                nc.vector.match_replace(out=pun[:], in_to_replace=vals0[:],
                                        in_values=sim2[:], imm_value=-1e30)
                nc.vector.max(vals1[:], pun[:])
                nc.vector.max_index(idx1[:], vals1[:], pun[:])

                idxf = sb.tile([64, TOPK], F32, tag="idxf")
                nc.vector.tensor_copy(idxf[:, 0:8], idx0[:])
                nc.vector.tensor_copy(idxf[:, 8:16], idx1[:])

                selT_ps = sp.tile([TOPK, 64], F32, tag="selT")
                nc.tensor.transpose(selT_ps[:], idxf[:], id64[:])
                selT = sb.tile([TOPK, 64], I16, tag="selTsb")
                nc.vector.tensor_copy(selT[:], selT_ps[:])

                for g in range(8):
                    half = selT[:, 0:32] if g < 4 else selT[:, 32:64]
                    nc.sync.dma_start(kidx[16 * g:16 * (g + 1), :], half)
                    nc.sync.dma_start(vidx0[16 * g:16 * (g + 1), :], selT[:, 0:32])
                    nc.sync.dma_start(vidx1[16 * g:16 * (g + 1), :], selT[:, 32:64])

            # ---- QKV (bf16) ----
            qT = pp.tile([128, N], BF16)
            kT = pp.tile([128, NB, BLK], BF16)   # contiguous == [128, N]
            v0 = pp.tile([128, NB, 66], BF16)
            v1 = pp.tile([128, NB, 66], BF16)
            nc.vector.memset(v0[:, :, 64:66], 0.0)
            nc.vector.memset(v1[:, :, 64:66], 0.0)
            nc.vector.memset(v0[:, :, 64:65], 1.0)
            nc.vector.memset(v1[:, :, 64:65], 1.0)

            with tc.tile_pool(name="qkps", bufs=3, space="PSUM") as qp:
                for mt in (0, 1):
                    for nch in range(8):
                        ps = qp.tile([128, 512], F32, tag="qk")
                        for kc in range(KC):
                            nc.tensor.matmul(
                                ps[:], lhsT=wqkv_bf[:, kc, mt * 128:(mt + 1) * 128],
                                rhs=xbf[:, kc, nch * 512:(nch + 1) * 512],
                                start=(kc == 0), stop=(kc == KC - 1))
                        if mt == 0:
                            nc.scalar.copy(qT[:, nch * 512:(nch + 1) * 512], ps[:])
                        else:
                            nc.scalar.copy(
                                kT[:].rearrange("p a b -> p (a b)")[:, nch * 512:(nch + 1) * 512],
                                ps[:])

            with tc.tile_pool(name="vps", bufs=3, space="PSUM") as vp:
                for nt in range(NB):
                    ps = vp.tile([128, 128], F32, tag="v")
                    for kc in range(KC):
                        nc.tensor.matmul(ps[:], lhsT=xbf[:, kc, nt * 128:(nt + 1) * 128],
                                         rhs=wqkv_bf[:, kc, 256:384],
                                         start=(kc == 0), stop=(kc == KC - 1))
                    nc.vector.tensor_copy(v0[:, nt, 0:64], ps[:, 0:64])
                    nc.vector.tensor_copy(v1[:, nt, 0:64], ps[:, 64:128])

            # ---- main loop: sparse attention + chunked projection partials ----
            CHQ = 8                    # query blocks per projection chunk
            CHT = CHQ * BLK            # 512 tokens per chunk
            with tc.tile_pool(name="gather", bufs=4) as gp, \
                 tc.tile_pool(name="escore", bufs=8) as ep, \
                 tc.tile_pool(name="sps", bufs=3, space="PSUM") as spp, \
                 tc.tile_pool(name="ops", bufs=2, space="PSUM") as opp, \
                 tc.tile_pool(name="otp", bufs=2) as otp, \
                 tc.tile_pool(name="prout", bufs=4) as pr, \
                 tc.tile_pool(name="osb", bufs=3) as ob:

                def _emit_proj(c):
                    ot = otp.tile([128, CHT], BF16, tag="ot", name=f"ot_{c}")
                    nc.sync.dma_start_transpose(
                        ot[:], obounce.ap()[c * CHT:(c + 1) * CHT, :])
                    for m in range(KC):
                        pj = spp.tile([128, 1024], F32, tag="s", name=f"pj_{c}_{m}")
                        for s2 in range(CHT // 512):
                            nc.tensor.matmul(pj[:, s2 * 512:(s2 + 1) * 512],
                                             lhsT=projW_bf[:, m * 128:(m + 1) * 128],
                                             rhs=ot[:, s2 * 512:(s2 + 1) * 512],
                                             start=True, stop=True)
                        po = pr.tile([128, CHT], F32, tag="po", name=f"po_{c}_{m}")
                        nc.vector.tensor_scalar(po[:], pj[:, 0:CHT],
                                                projb_sb[:, m:m + 1], None,
                                                op0=mybir.AluOpType.add)
                        nc.sync.dma_start(
                            out_ext.ap()[m * 128:(m + 1) * 128, c * CHT:(c + 1) * CHT],
                            po[:])

                for qb in range(NB):
                    kg = gp.tile([128, TOPK, BLK], BF16, tag="kg")
                    nc.gpsimd.ap_gather(kg[:], kT[:], kidx[:, qb:qb + 1],
                                        channels=128, num_elems=NB, d=BLK, num_idxs=TOPK)
                    vg0 = gp.tile([128, TOPK, 66], BF16, tag="vg0")
                    nc.gpsimd.ap_gather(vg0[:], v0[:], vidx0[:, qb:qb + 1],
                                        channels=128, num_elems=NB, d=66, num_idxs=TOPK)
                    vg1 = gp.tile([128, TOPK, 66], BF16, tag="vg1")
                    nc.gpsimd.ap_gather(vg1[:], v1[:], vidx1[:, qb:qb + 1],
                                        channels=128, num_elems=NB, d=66, num_idxs=TOPK)

                    qcol = slice(qb * BLK, (qb + 1) * BLK)
                    etiles = [[None, None], [None, None]]
                    for half in range(2):
                        s0 = spp.tile([128, 1024], F32, tag="s", name=f"s0_{qb}_{half}")
                        s1 = spp.tile([128, 1024], F32, tag="s", name=f"s1_{qb}_{half}")
                        for jj in range(8):
                            j = half * 8 + jj
                            nc.tensor.matmul(s0[:, jj * 128:(jj + 1) * 128],
                                             lhsT=kg[0:64, j, :], rhs=qT[0:64, qcol],
                                             start=True, stop=True)
                            nc.tensor.matmul(s1[:, jj * 128:(jj + 1) * 128],
                                             lhsT=kg[64:128, j, :], rhs=qT[64:128, qcol],
                                             start=True, stop=True)
                        e0 = ep.tile([128, 1024], BF16, tag="e", name=f"e0_{qb}_{half}")
                        e1 = ep.tile([128, 1024], BF16, tag="e", name=f"e1_{qb}_{half}")
                        nc.scalar.activation(e0[:], s0[:],
                                             mybir.ActivationFunctionType.Exp, scale=SCALE)
                        nc.scalar.activation(e1[:], s1[:],
                                             mybir.ActivationFunctionType.Exp, scale=SCALE)
                        etiles[0][half] = e0
                        etiles[1][half] = e1

                    onorm = ob.tile([128, 2 * D], BF16, tag="onorm")
                    for h in range(HPC):
                        vg = vg0 if h == 0 else vg1
                        o_ps = opp.tile([128, D + 1], F32, tag="o")
                        for j in range(TOPK):
                            nc.tensor.matmul(o_ps[:],
                                             lhsT=etiles[h][j // 8][:, (j % 8) * 128:(j % 8 + 1) * 128],
                                             rhs=vg[:, j, 0:D + 1],
                                             start=(j == 0), stop=(j == TOPK - 1))
                        rec = ob.tile([128, 1], F32, tag="rec")
                        nc.vector.reciprocal(rec[:], o_ps[:, D:D + 1])
                        nc.vector.tensor_scalar(onorm[:, h * D:(h + 1) * D],
                                                o_ps[:, 0:D], rec[:], None,
                                                op0=mybir.AluOpType.mult)
                    nc.sync.dma_start(obounce.ap()[qb * BLK:(qb + 1) * BLK, :],
                                      onorm[:])
                    if qb % CHQ == CHQ - 1:
                        _emit_proj(qb // CHQ)

    nc.compile()
    return nc


def _prep_inputs(x, qkv_w, proj_w, proj_b):
    x = np.asarray(x, dtype=np.float32)
    qkv_w = np.asarray(qkv_w, dtype=np.float32)
    proj_w = np.asarray(proj_w, dtype=np.float32)
    proj_b = np.asarray(proj_b, dtype=np.float32)

    xT = np.ascontiguousarray(x[0].T)                      # [C, N]
    ident64 = np.eye(64, dtype=np.float32)
    zero_b = np.zeros((128, 8), dtype=np.float32)
    in_maps = []
    for i in range(NCORES):
        h0 = HPC * i
        rows = []
        for part in range(3):                              # q, k, v row groups
            base = part * C + h0 * D
            rows.append(qkv_w[base:base + HPC * D, :])
        wqkv = np.concatenate(rows, axis=0)                # [384, C]
        cslice = slice(i * 2 * D, (i + 1) * 2 * D)
        in_maps.append({
            "xT": xT,
            "wqkvT": np.ascontiguousarray(wqkv.T),         # [C, 384]
            # [c_local, j]: rows = this core's 128 c-dims, cols = all 1024 j
            "projWT": np.ascontiguousarray(proj_w[:, cslice].T),
            # bias only on core 0 (partials are summed on the host)
            "projb": (np.ascontiguousarray(proj_b.reshape(8, 128).T)
                      if i == 0 else zero_b),
            "ident64": ident64,
        })
    return in_maps


def kernel(x, qkv_w, proj_w, proj_b, _trace=False):
    if "nc" not in _CACHE:
        _CACHE["nc"] = _build()
    nc = _CACHE["nc"]
    in_maps = _prep_inputs(x, qkv_w, proj_w, proj_b)
    res = run_bass_kernel_spmd(nc, in_maps, core_ids=list(range(NCORES)),
                               trace=_trace)
    outT = res.results[0]["out"].astype(np.float32)
    for i in range(1, NCORES):
        outT += res.results[i]["out"]
    out = np.ascontiguousarray(outT.T).reshape(1, N, C).astype(np.float32)
    if _trace:
        _CACHE["last_exec_time_ns"] = res.exec_time_ns
        _CACHE["last_results"] = res
    return out


# revision 18
# speedup vs baseline: 1.6623x; 1.0155x over previous
"""Block-sparse attention (SageAttention-style mean-similarity top-k) on 8 TRN2 NeuronCores.

Sharding: 16 heads tensor-parallel across 8 cores (2 heads/core).
  - qkv weight column-sharded per core (its 2 heads' q/k/v rows, pre-transposed on host)
  - block selection + block-sparse attention fully local per head
  - proj weight row-sharded: each core computes the full-shape PARTIAL product
    o_local @ projW[:, c_slice].T (+ bias on core 0 only); the host unshard step
    sums the 8 partials (the row-parallel reduction).

Per-core device pipeline (bf16 compute, f32 selection):
  x^T f32 -> block sums (DVE) -> qm/km/sim (f32 PE) -> top-16 via max8/max_index
  qkv matmuls (bf16 PE), k kept d-major, v token-major with a ones column
  per query block: ap_gather (GPSIMD ucode, SBUF->SBUF) pulls the 16 selected
  k/v blocks; scores s^T = k_sel^T q (two heads packed in the 128x128 PE array
  via row groups); exp on ACT straight from PSUM; o = (e^T)^T v_sel with the
  gathered ones column yielding the softmax denominator; per-partition
  normalize; chunk-wise DMA-transpose + projection partials streamed out.
"""

import os
import sys

for _p in ("/opt/trn_rl_repo", "/root/.axon_site/_ro/trn_rl_repo"):
    if os.path.isdir(_p) and _p not in sys.path:
        sys.path.insert(0, _p)

import numpy as np

import concourse.bass as bass
import concourse.bacc as bacc
import concourse.tile as tile
import concourse.mybir as mybir
from concourse.bass_utils import run_bass_kernel_spmd
from concourse.library_config import ap_gather as ap_gather_lib

# problem constants
N = 4096          # sequence length
C = 1024          # model dim
H = 16            # heads
D = 64            # head dim
BLK = 128         # block size
NB = N // BLK     # 32 blocks
TOPK = 16         # int(0.5 * NB)
NCORES = 8
HPC = H // NCORES  # 2 heads per core
SCALE = D ** -0.5  # 0.125

F32 = mybir.dt.float32
BF16 = mybir.dt.bfloat16
I16 = mybir.dt.int16
U32 = mybir.dt.uint32

_CACHE = {}


def _build():
    nc = bacc.Bacc("TRN2", target_bir_lowering=False, debug=False,
                   num_devices=NCORES)

    KC = C // 128  # 8 contraction tiles

    xT = nc.dram_tensor("xT", [C, N], F32, kind="ExternalInput")
    wqkvT = nc.dram_tensor("wqkvT", [C, 3 * 2 * D], F32, kind="ExternalInput")
    projWT = nc.dram_tensor("projWT", [2 * D, C], F32, kind="ExternalInput")
    projb = nc.dram_tensor("projb", [128, KC], F32, kind="ExternalInput")
    ident64 = nc.dram_tensor("ident64", [64, 64], F32, kind="ExternalInput")
    out_ext = nc.dram_tensor("out", [C, N], F32, kind="ExternalOutput")

    obounce = nc.dram_tensor("obounce", [N, 2 * D], BF16)

    with tile.TileContext(nc) as tc:
        nc.gpsimd.load_library(ap_gather_lib)

        with tc.tile_pool(name="persist", bufs=1) as pp:
            # ---- weights ----
            wqkv_bf = pp.tile([128, KC, 384], BF16)
            nc.gpsimd.dma_start(
                wqkv_bf[:], wqkvT.ap().rearrange("(a p) m -> p a m", p=128))
            wqk_f32 = pp.tile([128, KC, 256], F32)
            nc.sync.dma_start(
                wqk_f32[:], wqkvT.ap().rearrange("(a p) m -> p a m", p=128)[:, :, 0:256])
            projW_bf = pp.tile([128, C], BF16)          # [c_local, j]
            nc.gpsimd.dma_start(projW_bf[:], projWT.ap())
            projb_sb = pp.tile([128, KC], F32)          # bias for j-tile m in col m
            nc.sync.dma_start(projb_sb[:], projb.ap())
            id64 = pp.tile([64, 64], F32)
            nc.sync.dma_start(id64[:], ident64.ap())

            # ---- x: one f32 read; DVE block sums, ACT cast -> xbf ----
            xbf = pp.tile([128, KC, N], BF16)
            xm = pp.tile([128, KC, NB], F32)
            with tc.tile_pool(name="xload", bufs=2) as xp:
                for kc in range(KC):
                    xf = xp.tile([128, N], F32, tag="xf")
                    nc.sync.dma_start(xf[:], xT.ap()[kc * 128:(kc + 1) * 128, :])
                    nc.vector.tensor_reduce(
                        xm[:, kc, :], xf[:].rearrange("p (b t) -> p b t", t=BLK),
                        axis=mybir.AxisListType.X, op=mybir.AluOpType.add)
                    nc.scalar.copy(xbf[:, kc, :], xf[:])

            # ---- block-mean similarity + top-k selection (f32) ----
            kidx = pp.tile([128, NB], I16)
            vidx0 = pp.tile([128, NB], I16)
            vidx1 = pp.tile([128, NB], I16)
            with tc.tile_pool(name="selps", bufs=2, space="PSUM") as sp, \
                 tc.tile_pool(name="selsb", bufs=2) as sb:
                qm_ps = sp.tile([128, NB], F32, tag="qkm")
                km_ps = sp.tile([128, NB], F32, tag="qkm")
                for kc in range(KC):
                    nc.tensor.matmul(qm_ps[:], lhsT=wqk_f32[:, kc, 0:128],
                                     rhs=xm[:, kc, :], start=(kc == 0), stop=(kc == KC - 1))
                for kc in range(KC):
                    nc.tensor.matmul(km_ps[:], lhsT=wqk_f32[:, kc, 128:256],
                                     rhs=xm[:, kc, :], start=(kc == 0), stop=(kc == KC - 1))
                qm_sb = sb.tile([128, NB], F32, tag="qm")
                km_sb = sb.tile([128, NB], F32, tag="km")
                nc.scalar.copy(qm_sb[:], qm_ps[:])
                nc.scalar.copy(km_sb[:], km_ps[:])

                sim_ps = sp.tile([64, NB], F32, tag="sim")
                for h in range(HPC):
                    nc.tensor.matmul(sim_ps[h * 32:(h + 1) * 32, :],
                                     lhsT=qm_sb[h * 64:(h + 1) * 64, :],
                                     rhs=km_sb[h * 64:(h + 1) * 64, :],
                                     start=True, stop=True)
                sim2 = sb.tile([64, NB], F32, tag="sim2")
                nc.vector.tensor_copy(sim2[:], sim_ps[:])

                vals0 = sb.tile([64, 8], F32, tag="v0")
                idx0 = sb.tile([64, 8], U32, tag="i0")
                pun = sb.tile([64, NB], F32, tag="pun")
                vals1 = sb.tile([64, 8], F32, tag="v1")
                idx1 = sb.tile([64, 8], U32, tag="i1")
                nc.vector.max(vals0[:], sim2[:])
                nc.vector.max_index(idx0[:], vals0[:], sim2[:])
                nc.vector.match_replace(out=pun[:], in_to_replace=vals0[:],
                                        in_values=sim2[:], imm_value=-1e30)
                nc.vector.max(vals1[:], pun[:])
                nc.vector.max_index(idx1[:], vals1[:], pun[:])

                idxf = sb.tile([64, TOPK], F32, tag="idxf")
                nc.vector.tensor_copy(idxf[:, 0:8], idx0[:])
                nc.vector.tensor_copy(idxf[:, 8:16], idx1[:])

                selT_ps = sp.tile([TOPK, 64], F32, tag="selT")
                nc.tensor.transpose(selT_ps[:], idxf[:], id64[:])
                selT = sb.tile([TOPK, 64], I16, tag="selTsb")
                nc.vector.tensor_copy(selT[:], selT_ps[:])

                for g in range(8):
                    half = selT[:, 0:32] if g < 4 else selT[:, 32:64]
                    nc.sync.dma_start(kidx[16 * g:16 * (g + 1), :], half)
                    nc.sync.dma_start(vidx0[16 * g:16 * (g + 1), :], selT[:, 0:32])
                    nc.sync.dma_start(vidx1[16 * g:16 * (g + 1), :], selT[:, 32:64])

            # ---- QKV (bf16) ----
            qT = pp.tile([128, N], BF16)
            kT = pp.tile([128, NB, BLK], BF16)   # contiguous == [128, N]
            v0 = pp.tile([128, NB, 66], BF16)
            v1 = pp.tile([128, NB, 66], BF16)
            nc.vector.memset(v0[:, :, 64:66], 0.0)
            nc.vector.memset(v1[:, :, 64:66], 0.0)
            nc.vector.memset(v0[:, :, 64:65], 1.0)
            nc.vector.memset(v1[:, :, 64:65], 1.0)

            with tc.tile_pool(name="qkps", bufs=3, space="PSUM") as qp:
                for mt in (0, 1):
                    for nch in range(8):
                        ps = qp.tile([128, 512], F32, tag="qk")
                        for kc in range(KC):
                            nc.tensor.matmul(
                                ps[:], lhsT=wqkv_bf[:, kc, mt * 128:(mt + 1) * 128],
                                rhs=xbf[:, kc, nch * 512:(nch + 1) * 512],
                                start=(kc == 0), stop=(kc == KC - 1))
                        if mt == 0:
                            nc.scalar.copy(qT[:, nch * 512:(nch + 1) * 512], ps[:])
                        else:
                            nc.scalar.copy(
                                kT[:].rearrange("p a b -> p (a b)")[:, nch * 512:(nch + 1) * 512],
                                ps[:])

            with tc.tile_pool(name="vps", bufs=3, space="PSUM") as vp:
                for nt in range(NB):
                    ps = vp.tile([128, 128], F32, tag="v")
                    for kc in range(KC):
                        nc.tensor.matmul(ps[:], lhsT=xbf[:, kc, nt * 128:(nt + 1) * 128],
                                         rhs=wqkv_bf[:, kc, 256:384],
                                         start=(kc == 0), stop=(kc == KC - 1))
                    nc.vector.tensor_copy(v0[:, nt, 0:64], ps[:, 0:64])
                    nc.vector.tensor_copy(v1[:, nt, 0:64], ps[:, 64:128])

            # ---- main loop: sparse attention + chunked projection partials ----
            CHQ = 8                    # query blocks per projection chunk
            CHT = CHQ * BLK            # 512 tokens per chunk
            with tc.tile_pool(name="gather", bufs=4) as gp, \
                 tc.tile_pool(name="escore", bufs=12) as ep, \
                 tc.tile_pool(name="sps", bufs=3, space="PSUM") as spp, \
                 tc.tile_pool(name="ops", bufs=2, space="PSUM") as opp, \
                 tc.tile_pool(name="otp", bufs=2) as otp, \
                 tc.tile_pool(name="prout", bufs=4) as pr, \
                 tc.tile_pool(name="osb", bufs=3) as ob:

                def _emit_proj(c):
                    ot = otp.tile([128, CHT], BF16, tag="ot", name=f"ot_{c}")
                    nc.sync.dma_start_transpose(
                        ot[:], obounce.ap()[c * CHT:(c + 1) * CHT, :])
                    for m in range(KC):
                        pj = spp.tile([128, 1024], F32, tag="s", name=f"pj_{c}_{m}")
                        for s2 in range(CHT // 512):
                            nc.tensor.matmul(pj[:, s2 * 512:(s2 + 1) * 512],
                                             lhsT=projW_bf[:, m * 128:(m + 1) * 128],
                                             rhs=ot[:, s2 * 512:(s2 + 1) * 512],
                                             start=True, stop=True)
                        po = pr.tile([128, CHT], F32, tag="po", name=f"po_{c}_{m}")
                        nc.vector.tensor_scalar(po[:], pj[:, 0:CHT],
                                                projb_sb[:, m:m + 1], None,
                                                op0=mybir.AluOpType.add)
                        nc.sync.dma_start(
                            out_ext.ap()[m * 128:(m + 1) * 128, c * CHT:(c + 1) * CHT],
                            po[:])

                state = {}

                def emit_scores(qb):
                    kg = gp.tile([128, TOPK, BLK], BF16, tag="kg",
                                 name=f"kg_{qb}")
                    nc.gpsimd.ap_gather(kg[:], kT[:], kidx[:, qb:qb + 1],
                                        channels=128, num_elems=NB, d=BLK, num_idxs=TOPK)
                    vg0 = gp.tile([128, TOPK, 66], BF16, tag="vg0",
                                  name=f"vg0_{qb}")
                    nc.gpsimd.ap_gather(vg0[:], v0[:], vidx0[:, qb:qb + 1],
                                        channels=128, num_elems=NB, d=66, num_idxs=TOPK)
                    vg1 = gp.tile([128, TOPK, 66], BF16, tag="vg1",
                                  name=f"vg1_{qb}")
                    nc.gpsimd.ap_gather(vg1[:], v1[:], vidx1[:, qb:qb + 1],
                                        channels=128, num_elems=NB, d=66, num_idxs=TOPK)

                    qcol = slice(qb * BLK, (qb + 1) * BLK)
                    etiles = [[None, None], [None, None]]
                    for half in range(2):
                        s0 = spp.tile([128, 1024], F32, tag="s", name=f"s0_{qb}_{half}")
                        s1 = spp.tile([128, 1024], F32, tag="s", name=f"s1_{qb}_{half}")
                        for jj in range(8):
                            j = half * 8 + jj
                            nc.tensor.matmul(s0[:, jj * 128:(jj + 1) * 128],
                                             lhsT=kg[0:64, j, :], rhs=qT[0:64, qcol],
                                             start=True, stop=True)
                            nc.tensor.matmul(s1[:, jj * 128:(jj + 1) * 128],
                                             lhsT=kg[64:128, j, :], rhs=qT[64:128, qcol],
                                             start=True, stop=True)
                        e0 = ep.tile([128, 1024], BF16, tag="e", name=f"e0_{qb}_{half}")
                        e1 = ep.tile([128, 1024], BF16, tag="e", name=f"e1_{qb}_{half}")
                        nc.scalar.activation(e0[:], s0[:],
                                             mybir.ActivationFunctionType.Exp, scale=SCALE)
                        nc.scalar.activation(e1[:], s1[:],
                                             mybir.ActivationFunctionType.Exp, scale=SCALE)
                        etiles[0][half] = e0
                        etiles[1][half] = e1
                    state[qb] = (etiles, vg0, vg1)

                def emit_o(qb):
                    etiles, vg0, vg1 = state.pop(qb)
                    onorm = ob.tile([128, 2 * D], BF16, tag="onorm",
                                    name=f"on_{qb}")
                    for h in range(HPC):
                        vg = vg0 if h == 0 else vg1
                        o_ps = opp.tile([128, D + 1], F32, tag="o",
                                        name=f"o_{qb}_{h}")
                        for j in range(TOPK):
                            nc.tensor.matmul(o_ps[:],
                                             lhsT=etiles[h][j // 8][:, (j % 8) * 128:(j % 8 + 1) * 128],
                                             rhs=vg[:, j, 0:D + 1],
                                             start=(j == 0), stop=(j == TOPK - 1))
                        rec = ob.tile([128, 1], F32, tag="rec", name=f"r_{qb}_{h}")
                        nc.vector.reciprocal(rec[:], o_ps[:, D:D + 1])
                        nc.vector.tensor_scalar(onorm[:, h * D:(h + 1) * D],
                                                o_ps[:, 0:D], rec[:], None,
                                                op0=mybir.AluOpType.mult)
                    nc.sync.dma_start(obounce.ap()[qb * BLK:(qb + 1) * BLK, :],
                                      onorm[:])
                    if qb % CHQ == CHQ - 1:
                        _emit_proj(qb // CHQ)

                # software pipeline: o-phase trails scores by one iteration so
                # the exp latency of qb never blocks the PE stream
                for qb in range(NB):
                    emit_scores(qb)
                    if qb >= 1:
                        emit_o(qb - 1)
                emit_o(NB - 1)

    nc.compile()
    return nc


def _prep_inputs(x, qkv_w, proj_w, proj_b):
    x = np.asarray(x, dtype=np.float32)
    qkv_w = np.asarray(qkv_w, dtype=np.float32)
    proj_w = np.asarray(proj_w, dtype=np.float32)
    proj_b = np.asarray(proj_b, dtype=np.float32)

    xT = np.ascontiguousarray(x[0].T)                      # [C, N]
    ident64 = np.eye(64, dtype=np.float32)
    zero_b = np.zeros((128, 8), dtype=np.float32)
    in_maps = []
    for i in range(NCORES):
        h0 = HPC * i
        rows = []
        for part in range(3):                              # q, k, v row groups
            base = part * C + h0 * D
            rows.append(qkv_w[base:base + HPC * D, :])
        wqkv = np.concatenate(rows, axis=0)                # [384, C]
        cslice = slice(i * 2 * D, (i + 1) * 2 * D)
        in_maps.append({
            "xT": xT,
            "wqkvT": np.ascontiguousarray(wqkv.T),         # [C, 384]
            # [c_local, j]: rows = this core's 128 c-dims, cols = all 1024 j
            "projWT": np.ascontiguousarray(proj_w[:, cslice].T),
            # bias only on core 0 (partials are summed on the host)
            "projb": (np.ascontiguousarray(proj_b.reshape(8, 128).T)
                      if i == 0 else zero_b),
            "ident64": ident64,
        })
    return in_maps


def kernel(x, qkv_w, proj_w, proj_b, _trace=False):
    if "nc" not in _CACHE:
        _CACHE["nc"] = _build()
    nc = _CACHE["nc"]
    in_maps = _prep_inputs(x, qkv_w, proj_w, proj_b)
    res = run_bass_kernel_spmd(nc, in_maps, core_ids=list(range(NCORES)),
                               trace=_trace)
    outT = res.results[0]["out"].astype(np.float32)
    for i in range(1, NCORES):
        outT += res.results[i]["out"]
    out = np.ascontiguousarray(outT.T).reshape(1, N, C).astype(np.float32)
    if _trace:
        _CACHE["last_exec_time_ns"] = res.exec_time_ns
        _CACHE["last_results"] = res
    return out


# revision 19
# speedup vs baseline: 1.6755x; 1.0079x over previous
"""Block-sparse attention (SageAttention-style mean-similarity top-k) on 8 TRN2 NeuronCores.

Sharding: 16 heads tensor-parallel across 8 cores (2 heads/core).
  - qkv weight column-sharded per core (its 2 heads' q/k/v rows, pre-transposed on host)
  - block selection + block-sparse attention fully local per head
  - proj weight row-sharded: each core computes the full-shape PARTIAL product
    o_local @ projW[:, c_slice].T (+ bias on core 0 only); the host unshard step
    sums the 8 partials (the row-parallel reduction).

Per-core device pipeline (bf16 compute, f32 selection):
  x^T f32 -> block sums (DVE) -> qm/km/sim (f32 PE) -> top-16 via max8/max_index
  qkv matmuls (bf16 PE), k kept d-major, v token-major with a ones column
  per query block: ap_gather (GPSIMD ucode, SBUF->SBUF) pulls the 16 selected
  k/v blocks; scores s^T = k_sel^T q (two heads packed in the 128x128 PE array
  via row groups); exp on ACT straight from PSUM; o = (e^T)^T v_sel with the
  gathered ones column yielding the softmax denominator; per-partition
  normalize; chunk-wise DMA-transpose + projection partials streamed out.
"""

import os
import sys

for _p in ("/opt/trn_rl_repo", "/root/.axon_site/_ro/trn_rl_repo"):
    if os.path.isdir(_p) and _p not in sys.path:
        sys.path.insert(0, _p)

import numpy as np

import concourse.bass as bass
import concourse.bacc as bacc
import concourse.tile as tile
import concourse.mybir as mybir
from concourse.bass_utils import run_bass_kernel_spmd
from concourse.library_config import ap_gather as ap_gather_lib

# problem constants
N = 4096          # sequence length
C = 1024          # model dim
H = 16            # heads
D = 64            # head dim
BLK = 128         # block size
NB = N // BLK     # 32 blocks
TOPK = 16         # int(0.5 * NB)
NCORES = 8
HPC = H // NCORES  # 2 heads per core
SCALE = D ** -0.5  # 0.125

F32 = mybir.dt.float32
BF16 = mybir.dt.bfloat16
I16 = mybir.dt.int16
U32 = mybir.dt.uint32

_CACHE = {}


def _build():
    nc = bacc.Bacc("TRN2", target_bir_lowering=False, debug=False,
                   num_devices=NCORES)

    KC = C // 128  # 8 contraction tiles

    xT = nc.dram_tensor("xT", [C, N], F32, kind="ExternalInput")
    wqkvT = nc.dram_tensor("wqkvT", [C, 3 * 2 * D], F32, kind="ExternalInput")
    projWT = nc.dram_tensor("projWT", [2 * D, C], F32, kind="ExternalInput")
    projb = nc.dram_tensor("projb", [128, KC], F32, kind="ExternalInput")
    ident64 = nc.dram_tensor("ident64", [64, 64], F32, kind="ExternalInput")
    out_ext = nc.dram_tensor("out", [C, N], F32, kind="ExternalOutput")

    obounce = nc.dram_tensor("obounce", [N, 2 * D], BF16)

    with tile.TileContext(nc) as tc:
        nc.gpsimd.load_library(ap_gather_lib)

        with tc.tile_pool(name="persist", bufs=1) as pp:
            # ---- weights ----
            wqkv_bf = pp.tile([128, KC, 384], BF16)
            nc.gpsimd.dma_start(
                wqkv_bf[:], wqkvT.ap().rearrange("(a p) m -> p a m", p=128))
            wqk_f32 = pp.tile([128, KC, 256], F32)
            nc.sync.dma_start(
                wqk_f32[:], wqkvT.ap().rearrange("(a p) m -> p a m", p=128)[:, :, 0:256])
            projW_bf = pp.tile([128, C], BF16)          # [c_local, j]
            nc.gpsimd.dma_start(projW_bf[:], projWT.ap())
            projb_sb = pp.tile([128, KC], F32)          # bias for j-tile m in col m
            nc.sync.dma_start(projb_sb[:], projb.ap())
            id64 = pp.tile([64, 64], F32)
            nc.sync.dma_start(id64[:], ident64.ap())

            # ---- x: one f32 read; DVE block sums, ACT cast -> xbf ----
            xbf = pp.tile([128, KC, N], BF16)
            xm = pp.tile([128, KC, NB], F32)
            with tc.tile_pool(name="xload", bufs=2) as xp:
                for kc in range(KC):
                    xf = xp.tile([128, N], F32, tag="xf")
                    nc.sync.dma_start(xf[:], xT.ap()[kc * 128:(kc + 1) * 128, :])
                    nc.vector.tensor_reduce(
                        xm[:, kc, :], xf[:].rearrange("p (b t) -> p b t", t=BLK),
                        axis=mybir.AxisListType.X, op=mybir.AluOpType.add)
                    nc.scalar.copy(xbf[:, kc, :], xf[:])

            # ---- block-mean similarity + top-k selection (f32) ----
            kidx = pp.tile([128, NB], I16)
            vidx0 = pp.tile([128, NB], I16)
            vidx1 = pp.tile([128, NB], I16)
            with tc.tile_pool(name="selps", bufs=2, space="PSUM") as sp, \
                 tc.tile_pool(name="selsb", bufs=2) as sb:
                qm_ps = sp.tile([128, NB], F32, tag="qkm")
                km_ps = sp.tile([128, NB], F32, tag="qkm")
                for kc in range(KC):
                    nc.tensor.matmul(qm_ps[:], lhsT=wqk_f32[:, kc, 0:128],
                                     rhs=xm[:, kc, :], start=(kc == 0), stop=(kc == KC - 1))
                for kc in range(KC):
                    nc.tensor.matmul(km_ps[:], lhsT=wqk_f32[:, kc, 128:256],
                                     rhs=xm[:, kc, :], start=(kc == 0), stop=(kc == KC - 1))
                qm_sb = sb.tile([128, NB], F32, tag="qm")
                km_sb = sb.tile([128, NB], F32, tag="km")
                nc.scalar.copy(qm_sb[:], qm_ps[:])
                nc.scalar.copy(km_sb[:], km_ps[:])

                sim_ps = sp.tile([64, NB], F32, tag="sim")
                for h in range(HPC):
                    nc.tensor.matmul(sim_ps[h * 32:(h + 1) * 32, :],
                                     lhsT=qm_sb[h * 64:(h + 1) * 64, :],
                                     rhs=km_sb[h * 64:(h + 1) * 64, :],
                                     start=True, stop=True)
                sim2 = sb.tile([64, NB], F32, tag="sim2")
                nc.vector.tensor_copy(sim2[:], sim_ps[:])

                vals0 = sb.tile([64, 8], F32, tag="v0")
                idx0 = sb.tile([64, 8], U32, tag="i0")
                pun = sb.tile([64, NB], F32, tag="pun")
                vals1 = sb.tile([64, 8], F32, tag="v1")
                idx1 = sb.tile([64, 8], U32, tag="i1")
                nc.vector.max(vals0[:], sim2[:])
                nc.vector.max_index(idx0[:], vals0[:], sim2[:])
                nc.vector.match_replace(out=pun[:], in_to_replace=vals0[:],
                                        in_values=sim2[:], imm_value=-1e30)
                nc.vector.max(vals1[:], pun[:])
                nc.vector.max_index(idx1[:], vals1[:], pun[:])

                idxf = sb.tile([64, TOPK], F32, tag="idxf")
                nc.vector.tensor_copy(idxf[:, 0:8], idx0[:])
                nc.vector.tensor_copy(idxf[:, 8:16], idx1[:])

                selT_ps = sp.tile([TOPK, 64], F32, tag="selT")
                nc.tensor.transpose(selT_ps[:], idxf[:], id64[:])
                selT = sb.tile([TOPK, 64], I16, tag="selTsb")
                nc.vector.tensor_copy(selT[:], selT_ps[:])

                for g in range(8):
                    half = selT[:, 0:32] if g < 4 else selT[:, 32:64]
                    nc.sync.dma_start(kidx[16 * g:16 * (g + 1), :], half)
                    nc.sync.dma_start(vidx0[16 * g:16 * (g + 1), :], selT[:, 0:32])
                    nc.sync.dma_start(vidx1[16 * g:16 * (g + 1), :], selT[:, 32:64])

            # ---- QKV (bf16) ----
            qT = pp.tile([128, N], BF16)
            kT = pp.tile([128, NB, BLK], BF16)   # contiguous == [128, N]
            v0 = pp.tile([128, NB, 66], BF16)
            v1 = pp.tile([128, NB, 66], BF16)
            nc.vector.memset(v0[:, :, 64:66], 0.0)
            nc.vector.memset(v1[:, :, 64:66], 0.0)
            nc.vector.memset(v0[:, :, 64:65], 1.0)
            nc.vector.memset(v1[:, :, 64:65], 1.0)

            with tc.tile_pool(name="qkps", bufs=3, space="PSUM") as qp:
                for mt in (0, 1):
                    for nch in range(8):
                        ps = qp.tile([128, 512], F32, tag="qk")
                        for kc in range(KC):
                            nc.tensor.matmul(
                                ps[:], lhsT=wqkv_bf[:, kc, mt * 128:(mt + 1) * 128],
                                rhs=xbf[:, kc, nch * 512:(nch + 1) * 512],
                                start=(kc == 0), stop=(kc == KC - 1))
                        if mt == 0:
                            nc.scalar.copy(qT[:, nch * 512:(nch + 1) * 512], ps[:])
                        else:
                            nc.scalar.copy(
                                kT[:].rearrange("p a b -> p (a b)")[:, nch * 512:(nch + 1) * 512],
                                ps[:])

            with tc.tile_pool(name="vps", bufs=3, space="PSUM") as vp:
                for nt in range(NB):
                    ps = vp.tile([128, 128], F32, tag="v")
                    for kc in range(KC):
                        nc.tensor.matmul(ps[:], lhsT=xbf[:, kc, nt * 128:(nt + 1) * 128],
                                         rhs=wqkv_bf[:, kc, 256:384],
                                         start=(kc == 0), stop=(kc == KC - 1))
                    nc.vector.tensor_copy(v0[:, nt, 0:64], ps[:, 0:64])
                    nc.vector.tensor_copy(v1[:, nt, 0:64], ps[:, 64:128])

            # ---- main loop: sparse attention + chunked projection partials ----
            CHQ = 8                    # query blocks per projection chunk
            CHT = CHQ * BLK            # 512 tokens per chunk
            with tc.tile_pool(name="gather", bufs=4) as gp, \
                 tc.tile_pool(name="escore", bufs=12) as ep, \
                 tc.tile_pool(name="sps", bufs=3, space="PSUM") as spp, \
                 tc.tile_pool(name="ops", bufs=2, space="PSUM") as opp, \
                 tc.tile_pool(name="otp", bufs=2) as otp, \
                 tc.tile_pool(name="prout", bufs=4) as pr, \
                 tc.tile_pool(name="osb", bufs=3) as ob:

                def _emit_proj(c):
                    ot = otp.tile([128, CHT], BF16, tag="ot", name=f"ot_{c}")
                    nc.sync.dma_start_transpose(
                        ot[:], obounce.ap()[c * CHT:(c + 1) * CHT, :])
                    for m in range(KC):
                        pj = spp.tile([128, 1024], F32, tag="s", name=f"pj_{c}_{m}")
                        for s2 in range(CHT // 512):
                            nc.tensor.matmul(pj[:, s2 * 512:(s2 + 1) * 512],
                                             lhsT=projW_bf[:, m * 128:(m + 1) * 128],
                                             rhs=ot[:, s2 * 512:(s2 + 1) * 512],
                                             start=True, stop=True)
                        po = pr.tile([128, CHT], F32, tag="po", name=f"po_{c}_{m}")
                        nc.vector.tensor_scalar(po[:], pj[:, 0:CHT],
                                                projb_sb[:, m:m + 1], None,
                                                op0=mybir.AluOpType.add)
                        nc.sync.dma_start(
                            out_ext.ap()[m * 128:(m + 1) * 128, c * CHT:(c + 1) * CHT],
                            po[:])

                state = {}

                def emit_scores(qb):
                    kg = gp.tile([128, TOPK, BLK], BF16, tag="kg",
                                 name=f"kg_{qb}")
                    nc.gpsimd.ap_gather(kg[:], kT[:], kidx[:, qb:qb + 1],
                                        channels=128, num_elems=NB, d=BLK, num_idxs=TOPK)
                    vg0 = gp.tile([128, TOPK, 66], BF16, tag="vg0",
                                  name=f"vg0_{qb}")
                    nc.gpsimd.ap_gather(vg0[:], v0[:], vidx0[:, qb:qb + 1],
                                        channels=128, num_elems=NB, d=66, num_idxs=TOPK)
                    vg1 = gp.tile([128, TOPK, 66], BF16, tag="vg1",
                                  name=f"vg1_{qb}")
                    nc.gpsimd.ap_gather(vg1[:], v1[:], vidx1[:, qb:qb + 1],
                                        channels=128, num_elems=NB, d=66, num_idxs=TOPK)

                    qcol = slice(qb * BLK, (qb + 1) * BLK)
                    etiles = [[None, None], [None, None]]
                    for half in range(2):
                        s0 = spp.tile([128, 1024], F32, tag="s", name=f"s0_{qb}_{half}")
                        s1 = spp.tile([128, 1024], F32, tag="s", name=f"s1_{qb}_{half}")
                        for jj in range(8):
                            j = half * 8 + jj
                            nc.tensor.matmul(s0[:, jj * 128:(jj + 1) * 128],
                                             lhsT=kg[0:64, j, :], rhs=qT[0:64, qcol],
                                             start=True, stop=True)
                            nc.tensor.matmul(s1[:, jj * 128:(jj + 1) * 128],
                                             lhsT=kg[64:128, j, :], rhs=qT[64:128, qcol],
                                             start=True, stop=True)
                        e0 = ep.tile([128, 1024], BF16, tag="e", name=f"e0_{qb}_{half}")
                        e1 = ep.tile([128, 1024], BF16, tag="e", name=f"e1_{qb}_{half}")
                        nc.scalar.activation(e0[:], s0[:],
                                             mybir.ActivationFunctionType.Exp, scale=SCALE)
                        nc.scalar.activation(e1[:], s1[:],
                                             mybir.ActivationFunctionType.Exp, scale=SCALE)
                        etiles[0][half] = e0
                        etiles[1][half] = e1
                    onorm = ob.tile([128, 2 * D], BF16, tag="onorm",
                                    name=f"on_{qb}")
                    state[qb] = (etiles, vg0, vg1, onorm)

                def emit_o(qb, heads=(0, 1)):
                    if qb not in state:
                        return
                    etiles, vg0, vg1, onorm = state[qb]
                    for h in heads:
                        vg = vg0 if h == 0 else vg1
                        o_ps = opp.tile([128, D + 1], F32, tag="o",
                                        name=f"o_{qb}_{h}")
                        for j in range(TOPK):
                            nc.tensor.matmul(o_ps[:],
                                             lhsT=etiles[h][j // 8][:, (j % 8) * 128:(j % 8 + 1) * 128],
                                             rhs=vg[:, j, 0:D + 1],
                                             start=(j == 0), stop=(j == TOPK - 1))
                        rec = ob.tile([128, 1], F32, tag="rec", name=f"r_{qb}_{h}")
                        nc.vector.reciprocal(rec[:], o_ps[:, D:D + 1])
                        nc.vector.tensor_scalar(onorm[:, h * D:(h + 1) * D],
                                                o_ps[:, 0:D], rec[:], None,
                                                op0=mybir.AluOpType.mult)
                    if heads[-1] == 1:
                        state.pop(qb)
                        nc.sync.dma_start(obounce.ap()[qb * BLK:(qb + 1) * BLK, :],
                                          onorm[:])
                        if qb % CHQ == CHQ - 1:
                            _emit_proj(qb // CHQ)

                # software pipeline: o-phase trails scores by one iteration so
                # the exp latency of qb never blocks the PE stream
                for qb in range(NB):
                    emit_scores(qb)
                    emit_o(qb - 1, heads=(0,))
                    emit_o(qb - 1, heads=(1,))
                emit_o(NB - 1, heads=(0,))
                emit_o(NB - 1, heads=(1,))

    nc.compile()
    return nc


def _prep_inputs(x, qkv_w, proj_w, proj_b):
    x = np.asarray(x, dtype=np.float32)
    qkv_w = np.asarray(qkv_w, dtype=np.float32)
    proj_w = np.asarray(proj_w, dtype=np.float32)
    proj_b = np.asarray(proj_b, dtype=np.float32)

    xT = np.ascontiguousarray(x[0].T)                      # [C, N]
    ident64 = np.eye(64, dtype=np.float32)
    zero_b = np.zeros((128, 8), dtype=np.float32)
    in_maps = []
    for i in range(NCORES):
        h0 = HPC * i
        rows = []
        for part in range(3):                              # q, k, v row groups
            base = part * C + h0 * D
            rows.append(qkv_w[base:base + HPC * D, :])
        wqkv = np.concatenate(rows, axis=0)                # [384, C]
        cslice = slice(i * 2 * D, (i + 1) * 2 * D)
        in_maps.append({
            "xT": xT,
            "wqkvT": np.ascontiguousarray(wqkv.T),         # [C, 384]
            # [c_local, j]: rows = this core's 128 c-dims, cols = all 1024 j
            "projWT": np.ascontiguousarray(proj_w[:, cslice].T),
            # bias only on core 0 (partials are summed on the host)
            "projb": (np.ascontiguousarray(proj_b.reshape(8, 128).T)
                      if i == 0 else zero_b),
            "ident64": ident64,
        })
    return in_maps


def kernel(x, qkv_w, proj_w, proj_b, _trace=False):
    if "nc" not in _CACHE:
        _CACHE["nc"] = _build()
    nc = _CACHE["nc"]
    in_maps = _prep_inputs(x, qkv_w, proj_w, proj_b)
    res = run_bass_kernel_spmd(nc, in_maps, core_ids=list(range(NCORES)),
                               trace=_trace)
    outT = res.results[0]["out"].astype(np.float32)
    for i in range(1, NCORES):
        outT += res.results[i]["out"]
    out = np.ascontiguousarray(outT.T).reshape(1, N, C).astype(np.float32)
    if _trace:
        _CACHE["last_exec_time_ns"] = res.exec_time_ns
        _CACHE["last_results"] = res
    return out


# revision 20
# speedup vs baseline: 1.7082x; 1.0195x over previous
"""Block-sparse attention (SageAttention-style mean-similarity top-k) on 8 TRN2 NeuronCores.

Sharding: 16 heads tensor-parallel across 8 cores (2 heads/core).
  - qkv weight column-sharded per core (its 2 heads' q/k/v rows, pre-transposed on host)
  - block selection + block-sparse attention fully local per head
  - proj weight row-sharded: each core computes the full-shape PARTIAL product
    o_local @ projW[:, c_slice].T (+ bias on core 0 only); the host unshard step
    sums the 8 partials (the row-parallel reduction).

Per-core device pipeline (bf16 compute, f32 selection):
  x^T f32 -> block sums (DVE) -> qm/km/sim (f32 PE) -> top-16 via max8/max_index
  qkv matmuls (bf16 PE), k kept d-major, v token-major with a ones column
  per query block: ap_gather (GPSIMD ucode, SBUF->SBUF) pulls the 16 selected
  k/v blocks; scores s^T = k_sel^T q (two heads packed in the 128x128 PE array
  via row groups); exp on ACT straight from PSUM; o = (e^T)^T v_sel with the
  gathered ones column yielding the softmax denominator; per-partition
  normalize; chunk-wise DMA-transpose + projection partials streamed out.
"""

import os
import sys

for _p in ("/opt/trn_rl_repo", "/root/.axon_site/_ro/trn_rl_repo"):
    if os.path.isdir(_p) and _p not in sys.path:
        sys.path.insert(0, _p)

import numpy as np

import concourse.bass as bass
import concourse.bacc as bacc
import concourse.tile as tile
import concourse.mybir as mybir
from concourse.bass_utils import run_bass_kernel_spmd
from concourse.library_config import ap_gather as ap_gather_lib

# problem constants
N = 4096          # sequence length
C = 1024          # model dim
H = 16            # heads
D = 64            # head dim
BLK = 128         # block size
NB = N // BLK     # 32 blocks
TOPK = 16         # int(0.5 * NB)
NCORES = 8
HPC = H // NCORES  # 2 heads per core
SCALE = D ** -0.5  # 0.125

F32 = mybir.dt.float32
BF16 = mybir.dt.bfloat16
I16 = mybir.dt.int16
U32 = mybir.dt.uint32

_CACHE = {}


def _build():
    nc = bacc.Bacc("TRN2", target_bir_lowering=False, debug=False,
                   num_devices=NCORES)

    KC = C // 128  # 8 contraction tiles

    xT = nc.dram_tensor("xT", [C, N], F32, kind="ExternalInput")
    wqkvT = nc.dram_tensor("wqkvT", [C, 3 * 2 * D], F32, kind="ExternalInput")
    projWT = nc.dram_tensor("projWT", [2 * D, C], F32, kind="ExternalInput")
    projb = nc.dram_tensor("projb", [128, KC], F32, kind="ExternalInput")
    ident64 = nc.dram_tensor("ident64", [64, 64], F32, kind="ExternalInput")
    out_ext = nc.dram_tensor("out", [C, N], F32, kind="ExternalOutput")

    obounce = nc.dram_tensor("obounce", [N, 2 * D], BF16)

    with tile.TileContext(nc) as tc:
        nc.gpsimd.load_library(ap_gather_lib)

        with tc.tile_pool(name="persist", bufs=1) as pp:
            # ---- weights ----
            wqkv_bf = pp.tile([128, KC, 384], BF16)
            nc.gpsimd.dma_start(
                wqkv_bf[:], wqkvT.ap().rearrange("(a p) m -> p a m", p=128))
            wqk_f32 = pp.tile([128, KC, 256], F32)
            nc.sync.dma_start(
                wqk_f32[:], wqkvT.ap().rearrange("(a p) m -> p a m", p=128)[:, :, 0:256])
            projW_bf = pp.tile([128, C], BF16)          # [c_local, j]
            nc.gpsimd.dma_start(projW_bf[:], projWT.ap())
            projb_sb = pp.tile([128, KC], F32)          # bias for j-tile m in col m
            nc.sync.dma_start(projb_sb[:], projb.ap())
            id64 = pp.tile([64, 64], F32)
            nc.sync.dma_start(id64[:], ident64.ap())

            # ---- x: one f32 read; DVE block sums, ACT cast -> xbf ----
            xbf = pp.tile([128, KC, N], BF16)
            xm = pp.tile([128, KC, NB], F32)
            with tc.tile_pool(name="xload", bufs=2) as xp:
                for kc in range(KC):
                    xf = xp.tile([128, N], F32, tag="xf")
                    nc.sync.dma_start(xf[:], xT.ap()[kc * 128:(kc + 1) * 128, :])
                    nc.vector.tensor_reduce(
                        xm[:, kc, :], xf[:].rearrange("p (b t) -> p b t", t=BLK),
                        axis=mybir.AxisListType.X, op=mybir.AluOpType.add)
                    nc.scalar.copy(xbf[:, kc, :], xf[:])

            # ---- block-mean similarity + top-k selection (f32) ----
            kidx = pp.tile([128, NB], I16)
            vidx0 = pp.tile([128, NB], I16)
            vidx1 = pp.tile([128, NB], I16)
            with tc.tile_pool(name="selps", bufs=2, space="PSUM") as sp, \
                 tc.tile_pool(name="selsb", bufs=2) as sb:
                qm_ps = sp.tile([128, NB], F32, tag="qkm")
                km_ps = sp.tile([128, NB], F32, tag="qkm")
                for kc in range(KC):
                    nc.tensor.matmul(qm_ps[:], lhsT=wqk_f32[:, kc, 0:128],
                                     rhs=xm[:, kc, :], start=(kc == 0), stop=(kc == KC - 1))
                for kc in range(KC):
                    nc.tensor.matmul(km_ps[:], lhsT=wqk_f32[:, kc, 128:256],
                                     rhs=xm[:, kc, :], start=(kc == 0), stop=(kc == KC - 1))
                qm_sb = sb.tile([128, NB], F32, tag="qm")
                km_sb = sb.tile([128, NB], F32, tag="km")
                nc.scalar.copy(qm_sb[:], qm_ps[:])
                nc.scalar.copy(km_sb[:], km_ps[:])

                sim_ps = sp.tile([64, NB], F32, tag="sim")
                for h in range(HPC):
                    nc.tensor.matmul(sim_ps[h * 32:(h + 1) * 32, :],
                                     lhsT=qm_sb[h * 64:(h + 1) * 64, :],
                                     rhs=km_sb[h * 64:(h + 1) * 64, :],
                                     start=True, stop=True)
                sim2 = sb.tile([64, NB], F32, tag="sim2")
                nc.vector.tensor_copy(sim2[:], sim_ps[:])

                vals0 = sb.tile([64, 8], F32, tag="v0")
                idx0 = sb.tile([64, 8], U32, tag="i0")
                pun = sb.tile([64, NB], F32, tag="pun")
                vals1 = sb.tile([64, 8], F32, tag="v1")
                idx1 = sb.tile([64, 8], U32, tag="i1")
                nc.vector.max(vals0[:], sim2[:])
                nc.vector.max_index(idx0[:], vals0[:], sim2[:])
                nc.vector.match_replace(out=pun[:], in_to_replace=vals0[:],
                                        in_values=sim2[:], imm_value=-1e30)
                nc.vector.max(vals1[:], pun[:])
                nc.vector.max_index(idx1[:], vals1[:], pun[:])

                idxf = sb.tile([64, TOPK], F32, tag="idxf")
                nc.vector.tensor_copy(idxf[:, 0:8], idx0[:])
                nc.vector.tensor_copy(idxf[:, 8:16], idx1[:])

                selT_ps = sp.tile([TOPK, 64], F32, tag="selT")
                nc.tensor.transpose(selT_ps[:], idxf[:], id64[:])
                selT = sb.tile([TOPK, 64], I16, tag="selTsb")
                nc.vector.tensor_copy(selT[:], selT_ps[:])

                for g in range(8):
                    half = selT[:, 0:32] if g < 4 else selT[:, 32:64]
                    nc.sync.dma_start(kidx[16 * g:16 * (g + 1), :], half)
                    nc.sync.dma_start(vidx0[16 * g:16 * (g + 1), :], selT[:, 0:32])
                    nc.sync.dma_start(vidx1[16 * g:16 * (g + 1), :], selT[:, 32:64])

            # ---- QKV (bf16) ----
            qT = pp.tile([128, N], BF16)
            kT = pp.tile([128, NB, BLK], BF16)   # contiguous == [128, N]
            v0 = pp.tile([128, NB, 66], BF16)
            v1 = pp.tile([128, NB, 66], BF16)
            nc.vector.memset(v0[:, :, 64:66], 0.0)
            nc.vector.memset(v1[:, :, 64:66], 0.0)
            nc.vector.memset(v0[:, :, 64:65], 1.0)
            nc.vector.memset(v1[:, :, 64:65], 1.0)

            with tc.tile_pool(name="qkps", bufs=3, space="PSUM") as qp:
                for mt in (0, 1):
                    for nch in range(8):
                        ps = qp.tile([128, 512], F32, tag="qk")
                        for kc in range(KC):
                            nc.tensor.matmul(
                                ps[:], lhsT=wqkv_bf[:, kc, mt * 128:(mt + 1) * 128],
                                rhs=xbf[:, kc, nch * 512:(nch + 1) * 512],
                                start=(kc == 0), stop=(kc == KC - 1))
                        if mt == 0:
                            nc.scalar.copy(qT[:, nch * 512:(nch + 1) * 512], ps[:])
                        else:
                            nc.scalar.copy(
                                kT[:].rearrange("p a b -> p (a b)")[:, nch * 512:(nch + 1) * 512],
                                ps[:])

            with tc.tile_pool(name="vps", bufs=3, space="PSUM") as vp:
                for nt in range(NB):
                    ps = vp.tile([128, 128], F32, tag="v")
                    for kc in range(KC):
                        nc.tensor.matmul(ps[:], lhsT=xbf[:, kc, nt * 128:(nt + 1) * 128],
                                         rhs=wqkv_bf[:, kc, 256:384],
                                         start=(kc == 0), stop=(kc == KC - 1))
                    nc.vector.tensor_copy(v0[:, nt, 0:64], ps[:, 0:64])
                    nc.vector.tensor_copy(v1[:, nt, 0:64], ps[:, 64:128])

            # ---- main loop: sparse attention + chunked projection partials ----
            CHQ = 8                    # query blocks per projection chunk
            CHT = CHQ * BLK            # 512 tokens per chunk
            with tc.tile_pool(name="gather", bufs=6) as gp, \
                 tc.tile_pool(name="escore", bufs=12) as ep, \
                 tc.tile_pool(name="sps", bufs=3, space="PSUM") as spp, \
                 tc.tile_pool(name="ops", bufs=2, space="PSUM") as opp, \
                 tc.tile_pool(name="otp", bufs=2) as otp, \
                 tc.tile_pool(name="prout", bufs=4) as pr, \
                 tc.tile_pool(name="osb", bufs=3) as ob:

                def _emit_proj(c):
                    ot = otp.tile([128, CHT], BF16, tag="ot", name=f"ot_{c}")
                    nc.sync.dma_start_transpose(
                        ot[:], obounce.ap()[c * CHT:(c + 1) * CHT, :])
                    for m in range(KC):
                        pj = spp.tile([128, 1024], F32, tag="s", name=f"pj_{c}_{m}")
                        for s2 in range(CHT // 512):
                            nc.tensor.matmul(pj[:, s2 * 512:(s2 + 1) * 512],
                                             lhsT=projW_bf[:, m * 128:(m + 1) * 128],
                                             rhs=ot[:, s2 * 512:(s2 + 1) * 512],
                                             start=True, stop=True)
                        po = pr.tile([128, CHT], F32, tag="po", name=f"po_{c}_{m}")
                        nc.vector.tensor_scalar(po[:], pj[:, 0:CHT],
                                                projb_sb[:, m:m + 1], None,
                                                op0=mybir.AluOpType.add)
                        nc.sync.dma_start(
                            out_ext.ap()[m * 128:(m + 1) * 128, c * CHT:(c + 1) * CHT],
                            po[:])

                state = {}

                def emit_scores(qb):
                    kg = gp.tile([128, TOPK, BLK], BF16, tag="kg",
                                 name=f"kg_{qb}")
                    nc.gpsimd.ap_gather(kg[:], kT[:], kidx[:, qb:qb + 1],
                                        channels=128, num_elems=NB, d=BLK, num_idxs=TOPK)
                    vg0 = gp.tile([128, TOPK, 66], BF16, tag="vg0",
                                  name=f"vg0_{qb}")
                    nc.gpsimd.ap_gather(vg0[:], v0[:], vidx0[:, qb:qb + 1],
                                        channels=128, num_elems=NB, d=66, num_idxs=TOPK)
                    vg1 = gp.tile([128, TOPK, 66], BF16, tag="vg1",
                                  name=f"vg1_{qb}")
                    nc.gpsimd.ap_gather(vg1[:], v1[:], vidx1[:, qb:qb + 1],
                                        channels=128, num_elems=NB, d=66, num_idxs=TOPK)

                    qcol = slice(qb * BLK, (qb + 1) * BLK)
                    etiles = [[None, None], [None, None]]
                    for half in range(2):
                        s0 = spp.tile([128, 1024], F32, tag="s", name=f"s0_{qb}_{half}")
                        s1 = spp.tile([128, 1024], F32, tag="s", name=f"s1_{qb}_{half}")
                        for jj in range(8):
                            j = half * 8 + jj
                            nc.tensor.matmul(s0[:, jj * 128:(jj + 1) * 128],
                                             lhsT=kg[0:64, j, :], rhs=qT[0:64, qcol],
                                             start=True, stop=True)
                            nc.tensor.matmul(s1[:, jj * 128:(jj + 1) * 128],
                                             lhsT=kg[64:128, j, :], rhs=qT[64:128, qcol],
                                             start=True, stop=True)
                        e0 = ep.tile([128, 1024], BF16, tag="e", name=f"e0_{qb}_{half}")
                        e1 = ep.tile([128, 1024], BF16, tag="e", name=f"e1_{qb}_{half}")
                        nc.scalar.activation(e0[:], s0[:],
                                             mybir.ActivationFunctionType.Exp, scale=SCALE)
                        nc.scalar.activation(e1[:], s1[:],
                                             mybir.ActivationFunctionType.Exp, scale=SCALE)
                        etiles[0][half] = e0
                        etiles[1][half] = e1
                    onorm = ob.tile([128, 2 * D], BF16, tag="onorm",
                                    name=f"on_{qb}")
                    state[qb] = (etiles, vg0, vg1, onorm)

                def emit_o(qb, heads=(0, 1)):
                    if qb not in state:
                        return
                    etiles, vg0, vg1, onorm = state[qb]
                    for h in heads:
                        vg = vg0 if h == 0 else vg1
                        o_ps = opp.tile([128, D + 1], F32, tag="o",
                                        name=f"o_{qb}_{h}")
                        for j in range(TOPK):
                            nc.tensor.matmul(o_ps[:],
                                             lhsT=etiles[h][j // 8][:, (j % 8) * 128:(j % 8 + 1) * 128],
                                             rhs=vg[:, j, 0:D + 1],
                                             start=(j == 0), stop=(j == TOPK - 1))
                        rec = ob.tile([128, 1], F32, tag="rec", name=f"r_{qb}_{h}")
                        nc.vector.reciprocal(rec[:], o_ps[:, D:D + 1])
                        nc.vector.tensor_scalar(onorm[:, h * D:(h + 1) * D],
                                                o_ps[:, 0:D], rec[:], None,
                                                op0=mybir.AluOpType.mult)
                    if heads[-1] == 1:
                        state.pop(qb)
                        nc.sync.dma_start(obounce.ap()[qb * BLK:(qb + 1) * BLK, :],
                                          onorm[:])
                        if qb % CHQ == CHQ - 1:
                            _emit_proj(qb // CHQ)

                # software pipeline: o-phase trails scores by one iteration so
                # the exp latency of qb never blocks the PE stream
                for qb in range(NB):
                    emit_scores(qb)
                    emit_o(qb - 1, heads=(0,))
                    emit_o(qb - 1, heads=(1,))
                emit_o(NB - 1, heads=(0,))
                emit_o(NB - 1, heads=(1,))

    nc.compile()
    return nc


def _prep_inputs(x, qkv_w, proj_w, proj_b):
    x = np.asarray(x, dtype=np.float32)
    qkv_w = np.asarray(qkv_w, dtype=np.float32)
    proj_w = np.asarray(proj_w, dtype=np.float32)
    proj_b = np.asarray(proj_b, dtype=np.float32)

    xT = np.ascontiguousarray(x[0].T)                      # [C, N]
    ident64 = np.eye(64, dtype=np.float32)
    zero_b = np.zeros((128, 8), dtype=np.float32)
    in_maps = []
    for i in range(NCORES):
        h0 = HPC * i
        rows = []
        for part in range(3):                              # q, k, v row groups
            base = part * C + h0 * D
            rows.append(qkv_w[base:base + HPC * D, :])
        wqkv = np.concatenate(rows, axis=0)                # [384, C]
        cslice = slice(i * 2 * D, (i + 1) * 2 * D)
        in_maps.append({
            "xT": xT,
            "wqkvT": np.ascontiguousarray(wqkv.T),         # [C, 384]
            # [c_local, j]: rows = this core's 128 c-dims, cols = all 1024 j
            "projWT": np.ascontiguousarray(proj_w[:, cslice].T),
            # bias only on core 0 (partials are summed on the host)
            "projb": (np.ascontiguousarray(proj_b.reshape(8, 128).T)
                      if i == 0 else zero_b),
            "ident64": ident64,
        })
    return in_maps


def kernel(x, qkv_w, proj_w, proj_b, _trace=False):
    if "nc" not in _CACHE:
        _CACHE["nc"] = _build()
    nc = _CACHE["nc"]
    in_maps = _prep_inputs(x, qkv_w, proj_w, proj_b)
    res = run_bass_kernel_spmd(nc, in_maps, core_ids=list(range(NCORES)),
                               trace=_trace)
    outT = res.results[0]["out"].astype(np.float32)
    for i in range(1, NCORES):
        outT += res.results[i]["out"]
    out = np.ascontiguousarray(outT.T).reshape(1, N, C).astype(np.float32)
    if _trace:
        _CACHE["last_exec_time_ns"] = res.exec_time_ns
        _CACHE["last_results"] = res
    return out
